# revision 21
# baseline (speedup 1.0000x reference)
"""MoE BERT layer (nn_MoEBertLayer) on 8 Trainium2 NeuronCores.

Sharding: pure data parallel. B=8 samples -> 1 sample per core. The MoE
routing (per-sample expert selection) is done on the host: each core's
input map carries the weights of the expert its sample routed to, packed
into matmul-friendly tile layouts. The device kernel is a dense BERT
layer for a single [512, 768] sample. No collectives.

Kernel layout strategy (per core, S=512, H=768, FF=3072, NH=12, DH=64):
  - hidden_states shipped in both [S,H] (residual/LN side) and [H,S]
    (matmul lhsT side) layouts.
  - QT/KT computed directly in [H,S] layout (out = Wq[:,m]^T @ xT).
  - V computed in [S,H] layout, with a constant ones column appended per
    head (width 65): the attention-context matmul
    ctxU_h^T = [V_h | 1]^T @ exp(scores_h^T) then yields the softmax
    denominator as its last row for free.
  - scores_h^T = K_h Q_h^T computed per head ([Sk,Sq] layout), exp via
    ScalarE with the 1/sqrt(DH) scale fused; softmax normalization is
    applied to ctxU^T (768x512 elements instead of 12x512x512).
  - Wo/FFN2 outputs come out in [S,H] layout where LayerNorm reductions
    are free-dim reductions (bn_stats/bn_aggr).
  - One on-chip transpose x1 -> x1T (24 PE transposes) feeds the FFN.
"""

import os
import sys
import numpy as np
from contextlib import ExitStack

for _p in ("/opt/trn_rl_repo", os.path.expanduser("~/.axon_site/_ro/trn_rl_repo")):
    if os.path.isdir(_p) and _p not in sys.path:
        sys.path.append(_p)

import concourse.bass as bass
import concourse.bacc as bacc
import concourse.tile as tile
from concourse import mybir
from concourse.masks import make_identity

F32 = mybir.dt.float32
AF = mybir.ActivationFunctionType

P = 128
S = 512           # sequence length (per sample)
H = 768           # hidden size
FF = 3072         # FFN intermediate
NH = 12           # attention heads
DH = 64           # head dim
HK = H // P       # 6
SQ = S // P       # 4
FK = FF // P      # 24
VW = DH + 1       # 65: V head block + ones column
N_CORES = 8
EPS = 1e-12


def _emit(ctx, tc, flags):
    nc = tc.nc
    (use_bq, use_bk, use_bv, use_bo, use_bi, use_bout,
     use_mask, use_ln1, use_ln2) = flags

    xT_d = nc.dram_tensor("xT", [H, S], F32, kind="ExternalInput")
    x_d = nc.dram_tensor("x", [S, H], F32, kind="ExternalInput")
    wq_d = nc.dram_tensor("wq", [HK, P, HK, P], F32, kind="ExternalInput")
    wk_d = nc.dram_tensor("wk", [HK, P, HK, P], F32, kind="ExternalInput")
    wv_d = nc.dram_tensor("wv", [HK, P, H], F32, kind="ExternalInput")
    wo_d = nc.dram_tensor("wo", [HK, P, H], F32, kind="ExternalInput")
    wi_d = nc.dram_tensor("wi", [FK, P, HK, P], F32, kind="ExternalInput")
    wout_d = nc.dram_tensor("wout", [FK, P, H], F32, kind="ExternalInput")
    out_d = nc.dram_tensor("out", [S, H], F32, kind="ExternalOutput")

    # optional inputs (general path; absent in the fast path)
    bq_d = nc.dram_tensor("bq", [P, HK], F32, kind="ExternalInput") if use_bq else None
    bk_d = nc.dram_tensor("bk", [P, HK], F32, kind="ExternalInput") if use_bk else None
    bv_d = nc.dram_tensor("bv", [H], F32, kind="ExternalInput") if use_bv else None
    bo_d = nc.dram_tensor("bo", [H], F32, kind="ExternalInput") if use_bo else None
    bi_d = nc.dram_tensor("bi", [P, FK], F32, kind="ExternalInput") if use_bi else None
    bout_d = nc.dram_tensor("bout", [H], F32, kind="ExternalInput") if use_bout else None
    msk_d = nc.dram_tensor("msk", [P, SQ], F32, kind="ExternalInput") if use_mask else None
    ln1g_d = nc.dram_tensor("ln1g", [H], F32, kind="ExternalInput") if use_ln1 else None
    ln1b_d = nc.dram_tensor("ln1b", [H], F32, kind="ExternalInput") if use_ln1 else None
    ln2g_d = nc.dram_tensor("ln2g", [H], F32, kind="ExternalInput") if use_ln2 else None
    ln2b_d = nc.dram_tensor("ln2b", [H], F32, kind="ExternalInput") if use_ln2 else None

    def bcast_dram_row(dram_ap, parts=P):
        # DRAM [N] -> partition-broadcast [parts, N] AP for DMA
        return bass.AP(tensor=dram_ap.tensor, offset=dram_ap.offset,
                       ap=[[0, parts]] + list(dram_ap.ap))

    # ---------------- pools: whole-kernel lifetime ----------------
    const = ctx.enter_context(tc.tile_pool(name="const", bufs=1))
    outp = ctx.enter_context(tc.tile_pool(name="outp", bufs=2))
    wsmall = ctx.enter_context(tc.tile_pool(name="wsmall", bufs=4))
    wbig = ctx.enter_context(tc.tile_pool(name="wbig", bufs=6))
    smalls = ctx.enter_context(tc.tile_pool(name="smalls", bufs=4))

    ident = const.tile([P, P], F32)
    make_identity(nc, ident)
    eps_t = const.tile([P, 1], F32)
    nc.vector.memset(eps_t, EPS)

    bq_sb = bk_sb = bi_sb = None
    bv_bc = bo_bc = bout_bc = msk_sb = None
    ln1g_bc = ln1b_bc = ln2g_bc = ln2b_bc = None
    if use_bq:
        bq_sb = const.tile([P, HK], F32)
        nc.sync.dma_start(out=bq_sb, in_=bq_d[:])
    if use_bk:
        bk_sb = const.tile([P, HK], F32)
        nc.sync.dma_start(out=bk_sb, in_=bk_d[:])
    if use_bi:
        bi_sb = const.tile([P, FK], F32)
        nc.sync.dma_start(out=bi_sb, in_=bi_d[:])
    if use_bv:
        bv_bc = const.tile([P, H], F32)
        nc.sync.dma_start(out=bv_bc, in_=bcast_dram_row(bv_d[:]))
    if use_bo:
        bo_bc = const.tile([P, H], F32)
        nc.sync.dma_start(out=bo_bc, in_=bcast_dram_row(bo_d[:]))
    if use_bout:
        bout_bc = const.tile([P, H], F32)
        nc.sync.dma_start(out=bout_bc, in_=bcast_dram_row(bout_d[:]))
    if use_mask:
        msk_sb = const.tile([P, SQ], F32)
        nc.sync.dma_start(out=msk_sb, in_=msk_d[:])
    if use_ln1:
        ln1g_bc = const.tile([P, H], F32)
        nc.sync.dma_start(out=ln1g_bc, in_=bcast_dram_row(ln1g_d[:]))
        ln1b_bc = const.tile([P, H], F32)
        nc.sync.dma_start(out=ln1b_bc, in_=bcast_dram_row(ln1b_d[:]))
    if use_ln2:
        ln2g_bc = const.tile([P, H], F32)
        nc.sync.dma_start(out=ln2g_bc, in_=bcast_dram_row(ln2g_d[:]))
        ln2b_bc = const.tile([P, H], F32)
        nc.sync.dma_start(out=ln2b_bc, in_=bcast_dram_row(ln2b_d[:]))

    # layer-norm core: reads `a` [P,H] (SBUF), writes `dst` [P,H]
    def layernorm(a, dst, g_bc, b_bc, use_gb):
        st = smalls.tile([P, 3, 6], F32, tag="lnst")
        a3 = a.rearrange("p (n f) -> p n f", f=256)
        for sg in range(3):
            nc.vector.bn_stats(out=st[:, sg, :], in_=a3[:, sg, :])
        mv = smalls.tile([P, 2], F32, tag="lnmv")
        nc.vector.bn_aggr(out=mv, in_=st)
        sd = smalls.tile([P, 1], F32, tag="lnsd")
        nc.scalar.activation(sd, mv[:, 1:2], AF.Sqrt, bias=eps_t)
        rsig = smalls.tile([P, 1], F32, tag="lnrs")
        nc.vector.reciprocal(rsig, sd)
        nm = smalls.tile([P, 1], F32, tag="lnnm")
        nc.vector.tensor_mul(nm, mv[:, 0:1], rsig)
        nc.vector.tensor_scalar_mul(nm, nm, -1.0)
        nc.scalar.activation(dst, a, AF.Identity, bias=nm, scale=rsig)
        if use_gb:
            nc.vector.tensor_mul(dst, dst, g_bc)
            nc.vector.tensor_add(dst, dst, b_bc)

    # ---------------- mid-lifetime activations ----------------
    act1 = ctx.enter_context(tc.tile_pool(name="act1", bufs=1))
    x1_sb = act1.tile([P, SQ, H], F32)      # LN1 output [S,H]
    x1t_sb = act1.tile([P, HK, S], F32)     # x1 transposed [H,S]

    a_pool = ctx.enter_context(tc.tile_pool(name="a_pool", bufs=2))

    with ExitStack() as phase_ab:
        actA = phase_ab.enter_context(tc.tile_pool(name="actA", bufs=1))
        x_sb = actA.tile([P, SQ, H], F32)
        xT_sb = actA.tile([P, HK, S], F32)
        qt_sb = actA.tile([P, HK, S], F32)
        kt_sb = actA.tile([P, HK, S], F32)
        vt_sb = actA.tile([P, SQ, NH * VW], F32)
        ctxt_sb = actA.tile([P, HK, S], F32)

        for m in range(SQ):
            nc.sync.dma_start(out=x_sb[:, m, :], in_=x_d[m * P:(m + 1) * P, :])
        for m in range(HK):
            nc.sync.dma_start(out=xT_sb[:, m, :], in_=xT_d[m * P:(m + 1) * P, :])
        ph_att = phase_ab.enter_context(ExitStack())
        psAB = ph_att.enter_context(tc.tile_pool(name="psAB", bufs=1, space="PSUM"))
        expp = ph_att.enter_context(tc.tile_pool(name="expp", bufs=2))
        rbp = ph_att.enter_context(tc.tile_pool(name="rbp", bufs=2))

        # ---- QT / KT:  out[m] = W[:, m-block]^T @ xT  ([H,S] layout) ----
        for (w_d, dst, b_sb, useb) in ((wq_d, qt_sb, bq_sb, use_bq),
                                       (wk_d, kt_sb, bk_sb, use_bk)):
            for m in range(HK):
                w_t = wsmall.tile([P, HK, P], F32, tag="wsm")
                nc.sync.dma_start(out=w_t, in_=w_d[m])
                ps = psAB.tile([P, S], F32, tag="s512", bufs=3)
                for k in range(HK):
                    nc.tensor.matmul(ps, lhsT=w_t[:, k, :], rhs=xT_sb[:, k, :],
                                     start=(k == 0), stop=(k == HK - 1))
                if useb:
                    nc.scalar.activation(dst[:, m, :], ps, AF.Identity,
                                         bias=b_sb[:, m:m + 1])
                else:
                    nc.vector.tensor_copy(dst[:, m, :], ps)

        # ---- V in [S,H] layout with ones column per head -> vt_sb ----
        nc.vector.memset(
            vt_sb.rearrange("p m (h c) -> p m h c", c=VW)[:, :, :, DH:DH + 1], 1.0)
        wv_ts = []
        for k in range(HK):
            wv_t = wbig.tile([P, H], F32, tag="wbg")
            nc.sync.dma_start(out=wv_t, in_=wv_d[k])
            wv_ts.append(wv_t)
        for m in range(SQ):
            ps = psAB.tile([P, H], F32, tag="big", bufs=2)
            for k in range(HK):
                nc.tensor.matmul(ps[:, 0:512], lhsT=xT_sb[:, k, m * P:(m + 1) * P],
                                 rhs=wv_ts[k][:, 0:512],
                                 start=(k == 0), stop=(k == HK - 1))
            for k in range(HK):
                nc.tensor.matmul(ps[:, 512:H], lhsT=xT_sb[:, k, m * P:(m + 1) * P],
                                 rhs=wv_ts[k][:, 512:H],
                                 start=(k == 0), stop=(k == HK - 1))
            dst = vt_sb.rearrange("p m (h c) -> p m h c", c=VW)[:, m, :, 0:DH]
            src = ps.rearrange("p (h d) -> p h d", d=DH)
            if use_bv:
                nc.vector.tensor_add(
                    src, src, bv_bc.rearrange("p (h d) -> p h d", d=DH))
            nc.vector.tensor_copy(dst, src)

        # ---- per-head attention ----
        for h in range(NH):
            mt, pb = h // 2, 64 * (h % 2)
            est = expp.tile([P, SQ, S], F32, tag="est")
            for sk in range(SQ):
                ps_s = psAB.tile([P, S], F32, tag="s512", bufs=3)
                nc.tensor.matmul(
                    ps_s,
                    lhsT=kt_sb[pb:pb + DH, mt, sk * P:(sk + 1) * P],
                    rhs=qt_sb[pb:pb + DH, mt, :],
                    start=True, stop=True)
                if use_mask:
                    nc.scalar.activation(est[:, sk, :], ps_s, AF.Exp,
                                         bias=msk_sb[:, sk:sk + 1], scale=0.125)
                else:
                    nc.scalar.activation(est[:, sk, :], ps_s, AF.Exp, scale=0.125)
            ps_c = psAB.tile([P, S], F32, tag="ctx", bufs=1)
            for sk in range(SQ):
                nc.tensor.matmul(ps_c[0:VW, :],
                                 lhsT=vt_sb[:, sk, h * VW:(h + 1) * VW],
                                 rhs=est[:, sk, :],
                                 start=(sk == 0), stop=(sk == SQ - 1))
            nc.vector.tensor_copy(ctxt_sb[pb:pb + DH, mt, :], ps_c[0:DH, :])
            # softmax normalization of this head's ctxT rows: 1/sums
            # partition-broadcast on GpSimd (idle engine; exact on HW).
            rrow = smalls.tile([1, S], F32, tag="rrow")
            nc.vector.reciprocal(rrow, ps_c[DH:VW, :])
            rb = rbp.tile([P, S], F32, tag="rb")
            nc.gpsimd.partition_broadcast(rb, rrow)
            nc.vector.tensor_mul(ctxt_sb[pb:pb + DH, mt, :],
                                 ctxt_sb[pb:pb + DH, mt, :], rb[pb:pb + DH, :])

        # ---- Wo + residual + LN1 ; x1 transpose ----
        ph_att.close()
        with tc.tile_pool(name="psC", bufs=1, space="PSUM") as psC:
            wo_ts = []
            for k in range(HK):
                wo_t = wbig.tile([P, H], F32, tag="wbg")
                nc.sync.dma_start(out=wo_t, in_=wo_d[k])
                wo_ts.append(wo_t)
            for m in range(SQ):
                ps = psC.tile([P, H], F32, tag="cbig", bufs=2)
                for k in range(HK):
                    nc.tensor.matmul(ps[:, 0:512],
                                     lhsT=ctxt_sb[:, k, m * P:(m + 1) * P],
                                     rhs=wo_ts[k][:, 0:512],
                                     start=(k == 0), stop=(k == HK - 1))
                for k in range(HK):
                    nc.tensor.matmul(ps[:, 512:H],
                                     lhsT=ctxt_sb[:, k, m * P:(m + 1) * P],
                                     rhs=wo_ts[k][:, 512:H],
                                     start=(k == 0), stop=(k == HK - 1))
                a = a_pool.tile([P, H], F32, tag="a")
                nc.vector.tensor_add(a, ps, x_sb[:, m, :])
                if use_bo:
                    nc.vector.tensor_add(a, a, bo_bc)
                layernorm(a, x1_sb[:, m, :], ln1g_bc, ln1b_bc, use_ln1)
                for kb in range(HK):
                    ps_t = psC.tile([P, P], F32, tag="tr", bufs=3)
                    nc.tensor.transpose(
                        ps_t, x1_sb[:, m, kb * P:(kb + 1) * P], ident)
                    nc.vector.tensor_copy(
                        x1t_sb[:, kb, m * P:(m + 1) * P], ps_t)

    # ---- FFN ----
    with ExitStack() as phase_ffn:
        actF = phase_ffn.enter_context(tc.tile_pool(name="actF", bufs=1))
        hmidt_sb = actF.tile([P, FK, S], F32)

        with tc.tile_pool(name="psD", bufs=4, space="PSUM") as psD:
            for m in range(FK):
                wi_t = wsmall.tile([P, HK, P], F32, tag="wsm")
                nc.sync.dma_start(out=wi_t, in_=wi_d[m])
                ps = psD.tile([P, S], F32, tag="f1")
                for k in range(HK):
                    nc.tensor.matmul(ps, lhsT=wi_t[:, k, :], rhs=x1t_sb[:, k, :],
                                     start=(k == 0), stop=(k == HK - 1))
                if use_bi:
                    nc.scalar.activation(hmidt_sb[:, m, :], ps, AF.Gelu,
                                         bias=bi_sb[:, m:m + 1])
                else:
                    nc.scalar.activation(hmidt_sb[:, m, :], ps, AF.Gelu)

        with tc.tile_pool(name="psE", bufs=1, space="PSUM") as psE:
            ps_m = [psE.tile([P, H], F32, tag=f"f2_{m}", bufs=1, name=f"psE{m}")
                    for m in range(SQ)]
            for k in range(FK):
                wo_t = wbig.tile([P, H], F32, tag="wbg")
                nc.sync.dma_start(out=wo_t, in_=wout_d[k])
                for m in range(SQ):
                    nc.tensor.matmul(ps_m[m][:, 0:512],
                                     lhsT=hmidt_sb[:, k, m * P:(m + 1) * P],
                                     rhs=wo_t[:, 0:512],
                                     start=(k == 0), stop=(k == FK - 1))
                    nc.tensor.matmul(ps_m[m][:, 512:H],
                                     lhsT=hmidt_sb[:, k, m * P:(m + 1) * P],
                                     rhs=wo_t[:, 512:H],
                                     start=(k == 0), stop=(k == FK - 1))
            for m in range(SQ):
                a = a_pool.tile([P, H], F32, tag="a")
                nc.vector.tensor_add(a, ps_m[m], x1_sb[:, m, :])
                if use_bout:
                    nc.vector.tensor_add(a, a, bout_bc)
                o_t = outp.tile([P, H], F32, tag="out")
                layernorm(a, o_t, ln2g_bc, ln2b_bc, use_ln2)
                nc.sync.dma_start(out=out_d[m * P:(m + 1) * P, :], in_=o_t)


_NC_CACHE = {}


def build_nc(flags):
    key = tuple(flags)
    if key not in _NC_CACHE:
        nc = bacc.Bacc("TRN2")
        with ExitStack() as ctx:
            tc = ctx.enter_context(tile.TileContext(nc))
            _emit(ctx, tc, flags)
        nc.compile()
        _NC_CACHE[key] = nc
    return _NC_CACHE[key]


def _pack_lhsT(A, mt):
    # A [in, mt*P] -> [mt, P, in//P, P] tiles: out[m, p, k, f] = A[P*k+p, P*m+f]
    kt = A.shape[0] // P
    return np.ascontiguousarray(
        A.reshape(kt, P, mt, P).transpose(2, 1, 0, 3))


def kernel(**inputs):
    hs = np.ascontiguousarray(np.asarray(inputs["hidden_states"], dtype=np.float32))
    eidx = np.asarray(inputs["expert_idx"]).astype(np.int64)
    mask = np.asarray(inputs["attention_mask"], dtype=np.float32)
    Wq = np.asarray(inputs["Wq"], dtype=np.float32)
    bq = np.asarray(inputs["bq"], dtype=np.float32)
    Wk = np.asarray(inputs["Wk"], dtype=np.float32)
    bk = np.asarray(inputs["bk"], dtype=np.float32)
    Wv = np.asarray(inputs["Wv"], dtype=np.float32)
    bv = np.asarray(inputs["bv"], dtype=np.float32)
    Wo = np.asarray(inputs["Wo"], dtype=np.float32)
    bo = np.asarray(inputs["bo"], dtype=np.float32)
    ln1_g = np.asarray(inputs["ln1_g"], dtype=np.float32)
    ln1_b = np.asarray(inputs["ln1_b"], dtype=np.float32)
    Wi = np.asarray(inputs["Wi"], dtype=np.float32)
    bi = np.asarray(inputs["bi"], dtype=np.float32)
    Wout = np.asarray(inputs["Wout"], dtype=np.float32)
    bout = np.asarray(inputs["bout"], dtype=np.float32)
    ln2_g = np.asarray(inputs["ln2_g"], dtype=np.float32)
    ln2_b = np.asarray(inputs["ln2_b"], dtype=np.float32)

    B = hs.shape[0]
    assert hs.shape == (B, S, H) and B == N_CORES

    use_bq = bool(np.any(bq))
    use_bk = bool(np.any(bk))
    use_bv = bool(np.any(bv))
    use_bo = bool(np.any(bo))
    use_bi = bool(np.any(bi))
    use_bout = bool(np.any(bout))
    use_mask = bool(np.any(mask))
    use_ln1 = bool(np.any(ln1_g != 1.0) or np.any(ln1_b))
    use_ln2 = bool(np.any(ln2_g != 1.0) or np.any(ln2_b))
    flags = (use_bq, use_bk, use_bv, use_bo, use_bi, use_bout,
             use_mask, use_ln1, use_ln2)

    nc = build_nc(flags)

    in_maps = []
    for b in range(B):
        e = int(eidx[b])
        xb = hs[b]
        im = {
            "x": xb,
            "xT": np.ascontiguousarray(xb.T),
            "wq": _pack_lhsT(Wq[e], HK),
            "wk": _pack_lhsT(Wk[e], HK),
            "wv": np.ascontiguousarray(Wv[e].reshape(HK, P, H)),
            "wo": np.ascontiguousarray(Wo[e].reshape(HK, P, H)),
            "wi": _pack_lhsT(Wi[e], FK),
            "wout": np.ascontiguousarray(Wout[e].reshape(FK, P, H)),
        }
        if use_bq:
            im["bq"] = np.ascontiguousarray(bq[e].reshape(HK, P).T)
        if use_bk:
            im["bk"] = np.ascontiguousarray(bk[e].reshape(HK, P).T)
        if use_bv:
            im["bv"] = bv[e]
        if use_bo:
            im["bo"] = bo[e]
        if use_bi:
            im["bi"] = np.ascontiguousarray(bi[e].reshape(FK, P).T)
        if use_bout:
            im["bout"] = bout[e]
        if use_mask:
            im["msk"] = np.ascontiguousarray(mask[b, 0, 0, :].reshape(SQ, P).T)
        if use_ln1:
            im["ln1g"] = ln1_g
            im["ln1b"] = ln1_b
        if use_ln2:
            im["ln2g"] = ln2_g
            im["ln2b"] = ln2_b
        in_maps.append(im)

    from concourse.bass_utils import run_bass_kernel_spmd
    res = run_bass_kernel_spmd(nc, in_maps, core_ids=list(range(N_CORES)),
                               **RUN_KWARGS)
    global LAST_RESULTS
    LAST_RESULTS = res
    out = np.stack([res.results[b]["out"] for b in range(B)], axis=0)
    return out.astype(np.float32)


RUN_KWARGS = {}
LAST_RESULTS = None


if __name__ == "__main__":
    rng = np.random.default_rng(0)
    demo = {
        "hidden_states": rng.standard_normal((8, S, H), dtype=np.float32),
        "expert_idx": rng.integers(0, 4, size=8).astype(np.int32),
        "attention_mask": np.zeros((8, 1, 1, S), np.float32),
        "Wq": 0.02 * rng.standard_normal((4, H, H), dtype=np.float32),
        "bq": np.zeros((4, H), np.float32),
        "Wk": 0.02 * rng.standard_normal((4, H, H), dtype=np.float32),
        "bk": np.zeros((4, H), np.float32),
        "Wv": 0.02 * rng.standard_normal((4, H, H), dtype=np.float32),
        "bv": np.zeros((4, H), np.float32),
        "Wo": 0.02 * rng.standard_normal((4, H, H), dtype=np.float32),
        "bo": np.zeros((4, H), np.float32),
        "ln1_g": np.ones((H,), np.float32),
        "ln1_b": np.zeros((H,), np.float32),
        "Wi": 0.02 * rng.standard_normal((4, H, FF), dtype=np.float32),
        "bi": np.zeros((4, FF), np.float32),
        "Wout": 0.02 * rng.standard_normal((4, FF, H), dtype=np.float32),
        "bout": np.zeros((4, H), np.float32),
        "ln2_g": np.ones((H,), np.float32),
        "ln2_b": np.zeros((H,), np.float32),
    }
    out = kernel(**demo)
    print("out", out.shape, out.dtype, float(np.abs(out).mean()))


# revision 35
# speedup vs baseline: 1.5296x; 1.5296x over previous
"""MoE BERT layer (nn_MoEBertLayer) on 8 Trainium2 NeuronCores.

Sharding: pure data parallel. B=8 samples -> 1 sample per core. The MoE
routing (per-sample expert selection) is done on the host: each core's
input map carries the weights of the expert its sample routed to, packed
into matmul-friendly tile layouts. The device kernel is a dense BERT
layer for a single [512, 768] sample. No collectives.

Kernel layout strategy (per core, S=512, H=768, FF=3072, NH=12, DH=64):
  - hidden_states shipped in both [S,H] (residual/LN side) and [H,S]
    (matmul lhsT side) layouts.
  - QT/KT computed directly in [H,S] layout (out = Wq[:,m]^T @ xT).
  - V computed in [S,H] layout, with a constant ones column appended per
    head (width 65): the attention-context matmul
    ctxU_h^T = [V_h | 1]^T @ exp(scores_h^T) then yields the softmax
    denominator as its last row for free.
  - scores_h^T = K_h Q_h^T computed per head ([Sk,Sq] layout), exp via
    ScalarE with the 1/sqrt(DH) scale fused; softmax normalization is
    applied to ctxU^T (768x512 elements instead of 12x512x512).
  - Wo/FFN2 outputs come out in [S,H] layout where LayerNorm reductions
    are free-dim reductions (bn_stats/bn_aggr).
  - One on-chip transpose x1 -> x1T (24 PE transposes) feeds the FFN.
"""

import os
import sys
import numpy as np
from contextlib import ExitStack

for _p in ("/opt/trn_rl_repo", os.path.expanduser("~/.axon_site/_ro/trn_rl_repo")):
    if os.path.isdir(_p) and _p not in sys.path:
        sys.path.append(_p)

import concourse.bass as bass
import concourse.bacc as bacc
import concourse.tile as tile
from concourse import mybir
from concourse.masks import make_identity

F32 = mybir.dt.float32
F32R = mybir.dt.float32r
AF = mybir.ActivationFunctionType

P = 128
S = 512           # sequence length (per sample)
H = 768           # hidden size
FF = 3072         # FFN intermediate
NH = 12           # attention heads
DH = 64           # head dim
HK = H // P       # 6
SQ = S // P       # 4
FK = FF // P      # 24
VW = DH + 1       # 65: V head block + ones column
N_CORES = 8
EPS = 1e-12


def _emit(ctx, tc, flags):
    nc = tc.nc
    (use_bq, use_bk, use_bv, use_bo, use_bi, use_bout,
     use_mask, use_ln1, use_ln2) = flags

    xT_d = nc.dram_tensor("xT", [H, S], F32, kind="ExternalInput")
    x_d = nc.dram_tensor("x", [S, H], F32, kind="ExternalInput")
    wq_d = nc.dram_tensor("wq", [HK, P, HK, P], F32, kind="ExternalInput")
    wk_d = nc.dram_tensor("wk", [HK, P, HK, P], F32, kind="ExternalInput")
    wv_d = nc.dram_tensor("wv", [HK, P, H], F32, kind="ExternalInput")
    wo_d = nc.dram_tensor("wo", [HK, P, H], F32, kind="ExternalInput")
    wi_d = nc.dram_tensor("wi", [FK, P, HK, P], F32, kind="ExternalInput")
    wout_d = nc.dram_tensor("wout", [FK, P, H], F32, kind="ExternalInput")
    out_d = nc.dram_tensor("out", [S, H], F32, kind="ExternalOutput")

    # optional inputs (general path; absent in the fast path)
    bq_d = nc.dram_tensor("bq", [P, HK], F32, kind="ExternalInput") if use_bq else None
    bk_d = nc.dram_tensor("bk", [P, HK], F32, kind="ExternalInput") if use_bk else None
    bv_d = nc.dram_tensor("bv", [H], F32, kind="ExternalInput") if use_bv else None
    bo_d = nc.dram_tensor("bo", [H], F32, kind="ExternalInput") if use_bo else None
    bi_d = nc.dram_tensor("bi", [P, FK], F32, kind="ExternalInput") if use_bi else None
    bout_d = nc.dram_tensor("bout", [H], F32, kind="ExternalInput") if use_bout else None
    msk_d = nc.dram_tensor("msk", [P, SQ], F32, kind="ExternalInput") if use_mask else None
    ln1g_d = nc.dram_tensor("ln1g", [H], F32, kind="ExternalInput") if use_ln1 else None
    ln1b_d = nc.dram_tensor("ln1b", [H], F32, kind="ExternalInput") if use_ln1 else None
    ln2g_d = nc.dram_tensor("ln2g", [H], F32, kind="ExternalInput") if use_ln2 else None
    ln2b_d = nc.dram_tensor("ln2b", [H], F32, kind="ExternalInput") if use_ln2 else None

    def bcast_dram_row(dram_ap, parts=P):
        # DRAM [N] -> partition-broadcast [parts, N] AP for DMA
        return bass.AP(tensor=dram_ap.tensor, offset=dram_ap.offset,
                       ap=[[0, parts]] + list(dram_ap.ap))

    # ---------------- pools: whole-kernel lifetime ----------------
    const = ctx.enter_context(tc.tile_pool(name="const", bufs=1))
    outp = ctx.enter_context(tc.tile_pool(name="outp", bufs=2))
    wsmall = ctx.enter_context(tc.tile_pool(name="wsmall", bufs=4))
    wbig = ctx.enter_context(tc.tile_pool(name="wbig", bufs=6))
    wraw = ctx.enter_context(tc.tile_pool(name="wraw", bufs=3))
    smalls = ctx.enter_context(tc.tile_pool(name="smalls", bufs=4))

    # All matmul operands are float32r (single-pass PE mode, ~2^-12
    # rounding, 4x faster than true fp32). The BIR verifier requires a
    # rounding-capable producer, so DMA'd tensors go through a GpSimd
    # (otherwise idle) round-copy; on-chip operands are written as f32r
    # directly by their eviction op.
    def load_rounded(shape, dram_ap, tag, bufs=None):
        raw = wraw.tile(shape, F32, tag="wraw", name="wraw")
        nc.sync.dma_start(out=raw, in_=dram_ap)
        pool = wsmall if shape[-1] == P else wbig
        t = pool.tile(shape, F32R, tag=tag, name=tag, bufs=bufs)
        nc.gpsimd.tensor_copy(t, raw)
        return t

    ident = const.tile([P, P], F32)
    make_identity(nc, ident)
    eps_t = const.tile([P, 1], F32)
    nc.vector.memset(eps_t, EPS)

    bq_sb = bk_sb = bi_sb = None
    bv_bc = bo_bc = bout_bc = msk_sb = None
    ln1g_bc = ln1b_bc = ln2g_bc = ln2b_bc = None
    if use_bq:
        bq_sb = const.tile([P, HK], F32)
        nc.sync.dma_start(out=bq_sb, in_=bq_d[:])
    if use_bk:
        bk_sb = const.tile([P, HK], F32)
        nc.sync.dma_start(out=bk_sb, in_=bk_d[:])
    if use_bi:
        bi_sb = const.tile([P, FK], F32)
        nc.sync.dma_start(out=bi_sb, in_=bi_d[:])
    if use_bv:
        bv_bc = const.tile([P, H], F32)
        nc.sync.dma_start(out=bv_bc, in_=bcast_dram_row(bv_d[:]))
    if use_bo:
        bo_bc = const.tile([P, H], F32)
        nc.sync.dma_start(out=bo_bc, in_=bcast_dram_row(bo_d[:]))
    if use_bout:
        bout_bc = const.tile([P, H], F32)
        nc.sync.dma_start(out=bout_bc, in_=bcast_dram_row(bout_d[:]))
    if use_mask:
        msk_sb = const.tile([P, SQ], F32)
        nc.sync.dma_start(out=msk_sb, in_=msk_d[:])
    if use_ln1:
        ln1g_bc = const.tile([P, H], F32)
        nc.sync.dma_start(out=ln1g_bc, in_=bcast_dram_row(ln1g_d[:]))
        ln1b_bc = const.tile([P, H], F32)
        nc.sync.dma_start(out=ln1b_bc, in_=bcast_dram_row(ln1b_d[:]))
    if use_ln2:
        ln2g_bc = const.tile([P, H], F32)
        nc.sync.dma_start(out=ln2g_bc, in_=bcast_dram_row(ln2g_d[:]))
        ln2b_bc = const.tile([P, H], F32)
        nc.sync.dma_start(out=ln2b_bc, in_=bcast_dram_row(ln2b_d[:]))

    # layer-norm core: reads `a` [P,H] (SBUF), writes `dst` [P,H]
    def layernorm(a, dst, g_bc, b_bc, use_gb):
        st = smalls.tile([P, 3, 6], F32, tag="lnst")
        a3 = a.rearrange("p (n f) -> p n f", f=256)
        for sg in range(3):
            nc.vector.bn_stats(out=st[:, sg, :], in_=a3[:, sg, :])
        mv = smalls.tile([P, 2], F32, tag="lnmv")
        nc.vector.bn_aggr(out=mv, in_=st)
        sd = smalls.tile([P, 1], F32, tag="lnsd")
        nc.scalar.activation(sd, mv[:, 1:2], AF.Sqrt, bias=eps_t)
        rsig = smalls.tile([P, 1], F32, tag="lnrs")
        nc.vector.reciprocal(rsig, sd)
        nm = smalls.tile([P, 1], F32, tag="lnnm")
        nc.vector.tensor_mul(nm, mv[:, 0:1], rsig)
        nc.vector.tensor_scalar_mul(nm, nm, -1.0)
        nc.scalar.activation(dst, a, AF.Identity, bias=nm, scale=rsig)
        if use_gb:
            nc.vector.tensor_mul(dst, dst, g_bc)
            nc.vector.tensor_add(dst, dst, b_bc)

    # ---------------- mid-lifetime activations ----------------
    act1 = ctx.enter_context(tc.tile_pool(name="act1", bufs=1))
    x1_sb = act1.tile([P, SQ, H], F32)      # LN1 output [S,H]
    x1t_sb = act1.tile([P, HK, S], F32R)    # x1 transposed [H,S]

    a_pool = ctx.enter_context(tc.tile_pool(name="a_pool", bufs=2))

    with ExitStack() as phase_ab:
        actA = phase_ab.enter_context(tc.tile_pool(name="actA", bufs=1))
        x_sb = actA.tile([P, SQ, H], F32)
        xT_sb = actA.tile([P, HK, S], F32)
        xTr_sb = actA.tile([P, HK, S], F32R)
        qt_sb = actA.tile([P, HK, S], F32R)
        kt_sb = actA.tile([P, HK, S], F32R)
        vt_sb = actA.tile([P, SQ, NH * VW], F32R)
        ctxt_sb = actA.tile([P, HK, S], F32R)

        for m in range(SQ):
            nc.sync.dma_start(out=x_sb[:, m, :], in_=x_d[m * P:(m + 1) * P, :])
        for m in range(HK):
            nc.sync.dma_start(out=xT_sb[:, m, :], in_=xT_d[m * P:(m + 1) * P, :])
            nc.gpsimd.tensor_copy(xTr_sb[:, m, :], xT_sb[:, m, :])
        ph_att = phase_ab.enter_context(ExitStack())
        psAB = ph_att.enter_context(tc.tile_pool(name="psAB", bufs=1, space="PSUM"))
        expp = ph_att.enter_context(tc.tile_pool(name="expp", bufs=2))
        rbp = ph_att.enter_context(tc.tile_pool(name="rbp", bufs=2))

        # ---- QT / KT:  out[m] = W[:, m-block]^T @ xT  ([H,S] layout) ----
        for (w_d, dst, b_sb, useb) in ((wq_d, qt_sb, bq_sb, use_bq),
                                       (wk_d, kt_sb, bk_sb, use_bk)):
            for m in range(HK):
                w_t = load_rounded([P, HK, P], w_d[m], "wsm")
                ps = psAB.tile([P, S], F32, tag="s512", bufs=3)
                for k in range(HK):
                    nc.tensor.matmul(ps, lhsT=w_t[:, k, :], rhs=xTr_sb[:, k, :],
                                     start=(k == 0), stop=(k == HK - 1))
                if useb:
                    nc.scalar.activation(dst[:, m, :], ps, AF.Identity,
                                         bias=b_sb[:, m:m + 1])
                else:
                    nc.vector.tensor_copy(dst[:, m, :], ps)

        # ---- V in [S,H] layout with ones column per head -> vt_sb ----
        ones_t = const.tile([P, NH], F32)
        nc.vector.memset(ones_t, 1.0)
        vt_v = vt_sb.rearrange("p m (h c) -> p m h c", c=VW)
        for m in range(SQ):
            nc.vector.tensor_copy(
                vt_v[:, m, :, DH:DH + 1],
                ones_t.rearrange("p (h o) -> p h o", o=1))
        wv_ts = [load_rounded([P, H], wv_d[k], "wvo", bufs=HK) for k in range(HK)]
        for m in range(SQ):
            ps = psAB.tile([P, H], F32, tag="big", bufs=2)
            for k in range(HK):
                nc.tensor.matmul(ps[:, 0:512], lhsT=xTr_sb[:, k, m * P:(m + 1) * P],
                                 rhs=wv_ts[k][:, 0:512],
                                 start=(k == 0), stop=(k == HK - 1))
            for k in range(HK):
                nc.tensor.matmul(ps[:, 512:H], lhsT=xTr_sb[:, k, m * P:(m + 1) * P],
                                 rhs=wv_ts[k][:, 512:H],
                                 start=(k == 0), stop=(k == HK - 1))
            dst = vt_sb.rearrange("p m (h c) -> p m h c", c=VW)[:, m, :, 0:DH]
            src = ps.rearrange("p (h d) -> p h d", d=DH)
            if use_bv:
                nc.vector.tensor_add(
                    src, src, bv_bc.rearrange("p (h d) -> p h d", d=DH))
            nc.vector.tensor_copy(dst, src)

        # ---- per-head attention ----
        for h in range(NH):
            mt, pb = h // 2, 64 * (h % 2)
            est = expp.tile([P, SQ, S], F32R, tag="est")
            for sk in range(SQ):
                ps_s = psAB.tile([P, S], F32, tag="s512", bufs=3)
                nc.tensor.matmul(
                    ps_s,
                    lhsT=kt_sb[pb:pb + DH, mt, sk * P:(sk + 1) * P],
                    rhs=qt_sb[pb:pb + DH, mt, :],
                    start=True, stop=True)
                if use_mask:
                    nc.scalar.activation(est[:, sk, :], ps_s, AF.Exp,
                                         bias=msk_sb[:, sk:sk + 1], scale=0.125)
                else:
                    nc.scalar.activation(est[:, sk, :], ps_s, AF.Exp, scale=0.125)
            ps_c = psAB.tile([P, S], F32, tag="ctx", bufs=1)
            for sk in range(SQ):
                nc.tensor.matmul(ps_c[0:VW, :],
                                 lhsT=vt_sb[:, sk, h * VW:(h + 1) * VW],
                                 rhs=est[:, sk, :],
                                 start=(sk == 0), stop=(sk == SQ - 1))
            nc.vector.tensor_copy(ctxt_sb[pb:pb + DH, mt, :], ps_c[0:DH, :])
            # softmax normalization of this head's ctxT rows: 1/sums
            # partition-broadcast on GpSimd (idle engine; exact on HW).
            rrow = smalls.tile([1, S], F32, tag="rrow")
            nc.vector.reciprocal(rrow, ps_c[DH:VW, :])
            rb = rbp.tile([P, S], F32, tag="rb")
            nc.gpsimd.partition_broadcast(rb, rrow)
            nc.vector.tensor_mul(ctxt_sb[pb:pb + DH, mt, :],
                                 ctxt_sb[pb:pb + DH, mt, :], rb[pb:pb + DH, :])

        # ---- Wo + residual + LN1 ; x1 transpose ----
        ph_att.close()
        with tc.tile_pool(name="psC", bufs=1, space="PSUM") as psC:
            wo_ts = [load_rounded([P, H], wo_d[k], "wvo", bufs=HK)
                     for k in range(HK)]
            for m in range(SQ):
                ps = psC.tile([P, H], F32, tag="cbig", bufs=2)
                for k in range(HK):
                    nc.tensor.matmul(ps[:, 0:512],
                                     lhsT=ctxt_sb[:, k, m * P:(m + 1) * P],
                                     rhs=wo_ts[k][:, 0:512],
                                     start=(k == 0), stop=(k == HK - 1))
                for k in range(HK):
                    nc.tensor.matmul(ps[:, 512:H],
                                     lhsT=ctxt_sb[:, k, m * P:(m + 1) * P],
                                     rhs=wo_ts[k][:, 512:H],
                                     start=(k == 0), stop=(k == HK - 1))
                a = a_pool.tile([P, H], F32, tag="a")
                nc.vector.tensor_add(a, ps, x_sb[:, m, :])
                if use_bo:
                    nc.vector.tensor_add(a, a, bo_bc)
                layernorm(a, x1_sb[:, m, :], ln1g_bc, ln1b_bc, use_ln1)
                for kb in range(HK):
                    ps_t = psC.tile([P, P], F32, tag="tr", bufs=3)
                    nc.tensor.transpose(
                        ps_t, x1_sb[:, m, kb * P:(kb + 1) * P], ident)
                    nc.vector.tensor_copy(
                        x1t_sb[:, kb, m * P:(m + 1) * P], ps_t)

    # ---- FFN ----
    with ExitStack() as phase_ffn:
        actF = phase_ffn.enter_context(tc.tile_pool(name="actF", bufs=1))
        hmidt_sb = actF.tile([P, FK, S], F32R)

        with tc.tile_pool(name="psD", bufs=4, space="PSUM") as psD:
            for m in range(FK):
                wi_t = load_rounded([P, HK, P], wi_d[m], "wsm")
                ps = psD.tile([P, S], F32, tag="f1")
                for k in range(HK):
                    nc.tensor.matmul(ps, lhsT=wi_t[:, k, :], rhs=x1t_sb[:, k, :],
                                     start=(k == 0), stop=(k == HK - 1))
                if use_bi:
                    nc.scalar.activation(hmidt_sb[:, m, :], ps, AF.Gelu,
                                         bias=bi_sb[:, m:m + 1])
                else:
                    nc.scalar.activation(hmidt_sb[:, m, :], ps, AF.Gelu)

        with tc.tile_pool(name="psE", bufs=1, space="PSUM") as psE:
            ps_m = [psE.tile([P, H], F32, tag=f"f2_{m}", bufs=1, name=f"psE{m}")
                    for m in range(SQ)]
            for k in range(FK):
                wo_t = load_rounded([P, H], wout_d[k], "wout", bufs=3)
                for m in range(SQ):
                    nc.tensor.matmul(ps_m[m][:, 0:512],
                                     lhsT=hmidt_sb[:, k, m * P:(m + 1) * P],
                                     rhs=wo_t[:, 0:512],
                                     start=(k == 0), stop=(k == FK - 1))
                    nc.tensor.matmul(ps_m[m][:, 512:H],
                                     lhsT=hmidt_sb[:, k, m * P:(m + 1) * P],
                                     rhs=wo_t[:, 512:H],
                                     start=(k == 0), stop=(k == FK - 1))
            for m in range(SQ):
                a = a_pool.tile([P, H], F32, tag="a")
                nc.vector.tensor_add(a, ps_m[m], x1_sb[:, m, :])
                if use_bout:
                    nc.vector.tensor_add(a, a, bout_bc)
                o_t = outp.tile([P, H], F32, tag="out")
                layernorm(a, o_t, ln2g_bc, ln2b_bc, use_ln2)
                nc.sync.dma_start(out=out_d[m * P:(m + 1) * P, :], in_=o_t)


_NC_CACHE = {}


def build_nc(flags):
    key = tuple(flags)
    if key not in _NC_CACHE:
        nc = bacc.Bacc("TRN2")
        with ExitStack() as ctx:
            tc = ctx.enter_context(tile.TileContext(nc))
            _emit(ctx, tc, flags)
        nc.compile()
        _NC_CACHE[key] = nc
    return _NC_CACHE[key]


def _pack_lhsT(A, mt):
    # A [in, mt*P] -> [mt, P, in//P, P] tiles: out[m, p, k, f] = A[P*k+p, P*m+f]
    kt = A.shape[0] // P
    return np.ascontiguousarray(
        A.reshape(kt, P, mt, P).transpose(2, 1, 0, 3))


def kernel(**inputs):
    hs = np.ascontiguousarray(np.asarray(inputs["hidden_states"], dtype=np.float32))
    eidx = np.asarray(inputs["expert_idx"]).astype(np.int64)
    mask = np.asarray(inputs["attention_mask"], dtype=np.float32)
    Wq = np.asarray(inputs["Wq"], dtype=np.float32)
    bq = np.asarray(inputs["bq"], dtype=np.float32)
    Wk = np.asarray(inputs["Wk"], dtype=np.float32)
    bk = np.asarray(inputs["bk"], dtype=np.float32)
    Wv = np.asarray(inputs["Wv"], dtype=np.float32)
    bv = np.asarray(inputs["bv"], dtype=np.float32)
    Wo = np.asarray(inputs["Wo"], dtype=np.float32)
    bo = np.asarray(inputs["bo"], dtype=np.float32)
    ln1_g = np.asarray(inputs["ln1_g"], dtype=np.float32)
    ln1_b = np.asarray(inputs["ln1_b"], dtype=np.float32)
    Wi = np.asarray(inputs["Wi"], dtype=np.float32)
    bi = np.asarray(inputs["bi"], dtype=np.float32)
    Wout = np.asarray(inputs["Wout"], dtype=np.float32)
    bout = np.asarray(inputs["bout"], dtype=np.float32)
    ln2_g = np.asarray(inputs["ln2_g"], dtype=np.float32)
    ln2_b = np.asarray(inputs["ln2_b"], dtype=np.float32)

    B = hs.shape[0]
    assert hs.shape == (B, S, H) and B == N_CORES

    use_bq = bool(np.any(bq))
    use_bk = bool(np.any(bk))
    use_bv = bool(np.any(bv))
    use_bo = bool(np.any(bo))
    use_bi = bool(np.any(bi))
    use_bout = bool(np.any(bout))
    use_mask = bool(np.any(mask))
    use_ln1 = bool(np.any(ln1_g != 1.0) or np.any(ln1_b))
    use_ln2 = bool(np.any(ln2_g != 1.0) or np.any(ln2_b))
    flags = (use_bq, use_bk, use_bv, use_bo, use_bi, use_bout,
             use_mask, use_ln1, use_ln2)

    nc = build_nc(flags)

    in_maps = []
    for b in range(B):
        e = int(eidx[b])
        xb = hs[b]
        im = {
            "x": xb,
            "xT": np.ascontiguousarray(xb.T),
            "wq": _pack_lhsT(Wq[e], HK),
            "wk": _pack_lhsT(Wk[e], HK),
            "wv": np.ascontiguousarray(Wv[e].reshape(HK, P, H)),
            "wo": np.ascontiguousarray(Wo[e].reshape(HK, P, H)),
            "wi": _pack_lhsT(Wi[e], FK),
            "wout": np.ascontiguousarray(Wout[e].reshape(FK, P, H)),
        }
        if use_bq:
            im["bq"] = np.ascontiguousarray(bq[e].reshape(HK, P).T)
        if use_bk:
            im["bk"] = np.ascontiguousarray(bk[e].reshape(HK, P).T)
        if use_bv:
            im["bv"] = bv[e]
        if use_bo:
            im["bo"] = bo[e]
        if use_bi:
            im["bi"] = np.ascontiguousarray(bi[e].reshape(FK, P).T)
        if use_bout:
            im["bout"] = bout[e]
        if use_mask:
            im["msk"] = np.ascontiguousarray(mask[b, 0, 0, :].reshape(SQ, P).T)
        if use_ln1:
            im["ln1g"] = ln1_g
            im["ln1b"] = ln1_b
        if use_ln2:
            im["ln2g"] = ln2_g
            im["ln2b"] = ln2_b
        in_maps.append(im)

    from concourse.bass_utils import run_bass_kernel_spmd
    res = run_bass_kernel_spmd(nc, in_maps, core_ids=list(range(N_CORES)),
                               **RUN_KWARGS)
    global LAST_RESULTS
    LAST_RESULTS = res
    out = np.stack([res.results[b]["out"] for b in range(B)], axis=0)
    return out.astype(np.float32)


RUN_KWARGS = {}
LAST_RESULTS = None


if __name__ == "__main__":
    rng = np.random.default_rng(0)
    demo = {
        "hidden_states": rng.standard_normal((8, S, H), dtype=np.float32),
        "expert_idx": rng.integers(0, 4, size=8).astype(np.int32),
        "attention_mask": np.zeros((8, 1, 1, S), np.float32),
        "Wq": 0.02 * rng.standard_normal((4, H, H), dtype=np.float32),
        "bq": np.zeros((4, H), np.float32),
        "Wk": 0.02 * rng.standard_normal((4, H, H), dtype=np.float32),
        "bk": np.zeros((4, H), np.float32),
        "Wv": 0.02 * rng.standard_normal((4, H, H), dtype=np.float32),
        "bv": np.zeros((4, H), np.float32),
        "Wo": 0.02 * rng.standard_normal((4, H, H), dtype=np.float32),
        "bo": np.zeros((4, H), np.float32),
        "ln1_g": np.ones((H,), np.float32),
        "ln1_b": np.zeros((H,), np.float32),
        "Wi": 0.02 * rng.standard_normal((4, H, FF), dtype=np.float32),
        "bi": np.zeros((4, FF), np.float32),
        "Wout": 0.02 * rng.standard_normal((4, FF, H), dtype=np.float32),
        "bout": np.zeros((4, H), np.float32),
        "ln2_g": np.ones((H,), np.float32),
        "ln2_b": np.zeros((H,), np.float32),
    }
    out = kernel(**demo)
    print("out", out.shape, out.dtype, float(np.abs(out).mean()))


# revision 37
# speedup vs baseline: 2.1083x; 1.3783x over previous
"""MoE BERT layer (nn_MoEBertLayer) on 8 Trainium2 NeuronCores.

Sharding: pure data parallel. B=8 samples -> 1 sample per core. The MoE
routing (per-sample expert selection) is done on the host: each core's
input map carries the weights of the expert its sample routed to, packed
into matmul-friendly tile layouts. The device kernel is a dense BERT
layer for a single [512, 768] sample. No collectives.

Kernel layout strategy (per core, S=512, H=768, FF=3072, NH=12, DH=64):
  - hidden_states shipped in both [S,H] (residual/LN side) and [H,S]
    (matmul lhsT side) layouts.
  - QT/KT computed directly in [H,S] layout (out = Wq[:,m]^T @ xT).
  - V computed in [S,H] layout, with a constant ones column appended per
    head (width 65): the attention-context matmul
    ctxU_h^T = [V_h | 1]^T @ exp(scores_h^T) then yields the softmax
    denominator as its last row for free.
  - scores_h^T = K_h Q_h^T computed per head ([Sk,Sq] layout), exp via
    ScalarE with the 1/sqrt(DH) scale fused; softmax normalization is
    applied to ctxU^T (768x512 elements instead of 12x512x512).
  - Wo/FFN2 outputs come out in [S,H] layout where LayerNorm reductions
    are free-dim reductions (bn_stats/bn_aggr).
  - One on-chip transpose x1 -> x1T (24 PE transposes) feeds the FFN.
"""

import os
import sys
import numpy as np
from contextlib import ExitStack

for _p in ("/opt/trn_rl_repo", os.path.expanduser("~/.axon_site/_ro/trn_rl_repo")):
    if os.path.isdir(_p) and _p not in sys.path:
        sys.path.append(_p)

import concourse.bass as bass
import concourse.bacc as bacc
import concourse.tile as tile
from concourse import mybir
from concourse.masks import make_identity

F32 = mybir.dt.float32
F32R = mybir.dt.float32r
AF = mybir.ActivationFunctionType

P = 128
S = 512           # sequence length (per sample)
H = 768           # hidden size
FF = 3072         # FFN intermediate
NH = 12           # attention heads
DH = 64           # head dim
HK = H // P       # 6
SQ = S // P       # 4
FK = FF // P      # 24
VW = DH + 1       # 65: V head block + ones column
N_CORES = 8
EPS = 1e-12


def _emit(ctx, tc, flags):
    nc = tc.nc
    (use_bq, use_bk, use_bv, use_bo, use_bi, use_bout,
     use_mask, use_ln1, use_ln2) = flags

    xT_d = nc.dram_tensor("xT", [H, S], F32, kind="ExternalInput")
    x_d = nc.dram_tensor("x", [S, H], F32, kind="ExternalInput")
    wq_d = nc.dram_tensor("wq", [HK, P, HK, P], F32, kind="ExternalInput")
    wk_d = nc.dram_tensor("wk", [HK, P, HK, P], F32, kind="ExternalInput")
    wv_d = nc.dram_tensor("wv", [HK, P, H], F32, kind="ExternalInput")
    wo_d = nc.dram_tensor("wo", [HK, P, H], F32, kind="ExternalInput")
    wi_d = nc.dram_tensor("wi", [FK, P, HK, P], F32, kind="ExternalInput")
    wout_d = nc.dram_tensor("wout", [FK, P, H], F32, kind="ExternalInput")
    out_d = nc.dram_tensor("out", [S, H], F32, kind="ExternalOutput")

    # optional inputs (general path; absent in the fast path)
    bq_d = nc.dram_tensor("bq", [P, HK], F32, kind="ExternalInput") if use_bq else None
    bk_d = nc.dram_tensor("bk", [P, HK], F32, kind="ExternalInput") if use_bk else None
    bv_d = nc.dram_tensor("bv", [H], F32, kind="ExternalInput") if use_bv else None
    bo_d = nc.dram_tensor("bo", [H], F32, kind="ExternalInput") if use_bo else None
    bi_d = nc.dram_tensor("bi", [P, FK], F32, kind="ExternalInput") if use_bi else None
    bout_d = nc.dram_tensor("bout", [H], F32, kind="ExternalInput") if use_bout else None
    msk_d = nc.dram_tensor("msk", [P, SQ], F32, kind="ExternalInput") if use_mask else None
    ln1g_d = nc.dram_tensor("ln1g", [H], F32, kind="ExternalInput") if use_ln1 else None
    ln1b_d = nc.dram_tensor("ln1b", [H], F32, kind="ExternalInput") if use_ln1 else None
    ln2g_d = nc.dram_tensor("ln2g", [H], F32, kind="ExternalInput") if use_ln2 else None
    ln2b_d = nc.dram_tensor("ln2b", [H], F32, kind="ExternalInput") if use_ln2 else None

    def bcast_dram_row(dram_ap, parts=P):
        # DRAM [N] -> partition-broadcast [parts, N] AP for DMA
        return bass.AP(tensor=dram_ap.tensor, offset=dram_ap.offset,
                       ap=[[0, parts]] + list(dram_ap.ap))

    # ---------------- pools: whole-kernel lifetime ----------------
    const = ctx.enter_context(tc.tile_pool(name="const", bufs=1))
    outp = ctx.enter_context(tc.tile_pool(name="outp", bufs=2))
    wsmall = ctx.enter_context(tc.tile_pool(name="wsmall", bufs=4))
    wbig = ctx.enter_context(tc.tile_pool(name="wbig", bufs=6))
    wraw = ctx.enter_context(tc.tile_pool(name="wraw", bufs=3))
    smalls = ctx.enter_context(tc.tile_pool(name="smalls", bufs=4))

    # All matmul operands are float32r (single-pass PE mode, ~2^-12
    # rounding, 4x faster than true fp32). The BIR verifier requires a
    # rounding-capable producer, so DMA'd tensors go through a GpSimd
    # (otherwise idle) round-copy; on-chip operands are written as f32r
    # directly by their eviction op.
    def load_rounded(shape, dram_ap, tag, bufs=None, eng=None):
        raw = wraw.tile(shape, F32, tag="wraw", name="wraw")
        nc.sync.dma_start(out=raw, in_=dram_ap)
        pool = wsmall if shape[-1] == P else wbig
        t = pool.tile(shape, F32R, tag=tag, name=tag, bufs=bufs)
        (eng or nc.vector).tensor_copy(t, raw)
        return t

    ident = const.tile([P, P], F32)
    make_identity(nc, ident)
    eps_t = const.tile([P, 1], F32)
    nc.vector.memset(eps_t, EPS)

    bq_sb = bk_sb = bi_sb = None
    bv_bc = bo_bc = bout_bc = msk_sb = None
    ln1g_bc = ln1b_bc = ln2g_bc = ln2b_bc = None
    if use_bq:
        bq_sb = const.tile([P, HK], F32)
        nc.sync.dma_start(out=bq_sb, in_=bq_d[:])
    if use_bk:
        bk_sb = const.tile([P, HK], F32)
        nc.sync.dma_start(out=bk_sb, in_=bk_d[:])
    if use_bi:
        bi_sb = const.tile([P, FK], F32)
        nc.sync.dma_start(out=bi_sb, in_=bi_d[:])
    if use_bv:
        bv_bc = const.tile([P, H], F32)
        nc.sync.dma_start(out=bv_bc, in_=bcast_dram_row(bv_d[:]))
    if use_bo:
        bo_bc = const.tile([P, H], F32)
        nc.sync.dma_start(out=bo_bc, in_=bcast_dram_row(bo_d[:]))
    if use_bout:
        bout_bc = const.tile([P, H], F32)
        nc.sync.dma_start(out=bout_bc, in_=bcast_dram_row(bout_d[:]))
    if use_mask:
        msk_sb = const.tile([P, SQ], F32)
        nc.sync.dma_start(out=msk_sb, in_=msk_d[:])
    if use_ln1:
        ln1g_bc = const.tile([P, H], F32)
        nc.sync.dma_start(out=ln1g_bc, in_=bcast_dram_row(ln1g_d[:]))
        ln1b_bc = const.tile([P, H], F32)
        nc.sync.dma_start(out=ln1b_bc, in_=bcast_dram_row(ln1b_d[:]))
    if use_ln2:
        ln2g_bc = const.tile([P, H], F32)
        nc.sync.dma_start(out=ln2g_bc, in_=bcast_dram_row(ln2g_d[:]))
        ln2b_bc = const.tile([P, H], F32)
        nc.sync.dma_start(out=ln2b_bc, in_=bcast_dram_row(ln2b_d[:]))

    # layer-norm core: reads `a` [P,H] (SBUF), writes `dst` [P,H]
    def layernorm(a, dst, g_bc, b_bc, use_gb):
        st = smalls.tile([P, 3, 6], F32, tag="lnst")
        a3 = a.rearrange("p (n f) -> p n f", f=256)
        for sg in range(3):
            nc.vector.bn_stats(out=st[:, sg, :], in_=a3[:, sg, :])
        mv = smalls.tile([P, 2], F32, tag="lnmv")
        nc.vector.bn_aggr(out=mv, in_=st)
        sd = smalls.tile([P, 1], F32, tag="lnsd")
        nc.scalar.activation(sd, mv[:, 1:2], AF.Sqrt, bias=eps_t)
        rsig = smalls.tile([P, 1], F32, tag="lnrs")
        nc.vector.reciprocal(rsig, sd)
        nm = smalls.tile([P, 1], F32, tag="lnnm")
        nc.vector.tensor_mul(nm, mv[:, 0:1], rsig)
        nc.vector.tensor_scalar_mul(nm, nm, -1.0)
        nc.scalar.activation(dst, a, AF.Identity, bias=nm, scale=rsig)
        if use_gb:
            nc.vector.tensor_mul(dst, dst, g_bc)
            nc.vector.tensor_add(dst, dst, b_bc)

    # ---------------- mid-lifetime activations ----------------
    act1 = ctx.enter_context(tc.tile_pool(name="act1", bufs=1))
    x1_sb = act1.tile([P, SQ, H], F32)      # LN1 output [S,H]
    x1t_sb = act1.tile([P, HK, S], F32R)    # x1 transposed [H,S]

    a_pool = ctx.enter_context(tc.tile_pool(name="a_pool", bufs=2))

    with ExitStack() as phase_ab:
        actA = phase_ab.enter_context(tc.tile_pool(name="actA", bufs=1))
        x_sb = actA.tile([P, SQ, H], F32)
        xT_sb = actA.tile([P, HK, S], F32)
        xTr_sb = actA.tile([P, HK, S], F32R)
        qt_sb = actA.tile([P, HK, S], F32R)
        kt_sb = actA.tile([P, HK, S], F32R)
        vt_sb = actA.tile([P, SQ, NH * VW], F32R)
        ctxt_sb = actA.tile([P, HK, S], F32R)

        for m in range(SQ):
            nc.sync.dma_start(out=x_sb[:, m, :], in_=x_d[m * P:(m + 1) * P, :])
        for m in range(HK):
            nc.sync.dma_start(out=xT_sb[:, m, :], in_=xT_d[m * P:(m + 1) * P, :])
            nc.vector.tensor_copy(xTr_sb[:, m, :], xT_sb[:, m, :])
        ph_att = phase_ab.enter_context(ExitStack())
        psAB = ph_att.enter_context(tc.tile_pool(name="psAB", bufs=1, space="PSUM"))
        expp = ph_att.enter_context(tc.tile_pool(name="expp", bufs=2))
        rbp = ph_att.enter_context(tc.tile_pool(name="rbp", bufs=2))

        # ---- QT / KT:  out[m] = W[:, m-block]^T @ xT  ([H,S] layout) ----
        for (w_d, dst, b_sb, useb) in ((wq_d, qt_sb, bq_sb, use_bq),
                                       (wk_d, kt_sb, bk_sb, use_bk)):
            for m in range(HK):
                w_t = load_rounded([P, HK, P], w_d[m], "wsm")
                ps = psAB.tile([P, S], F32, tag="s512", bufs=3)
                for k in range(HK):
                    nc.tensor.matmul(ps, lhsT=w_t[:, k, :], rhs=xTr_sb[:, k, :],
                                     start=(k == 0), stop=(k == HK - 1))
                if useb:
                    nc.scalar.activation(dst[:, m, :], ps, AF.Identity,
                                         bias=b_sb[:, m:m + 1])
                else:
                    nc.vector.tensor_copy(dst[:, m, :], ps)

        # ---- V in [S,H] layout with ones column per head -> vt_sb ----
        ones_t = const.tile([P, NH], F32)
        nc.vector.memset(ones_t, 1.0)
        vt_v = vt_sb.rearrange("p m (h c) -> p m h c", c=VW)
        for m in range(SQ):
            nc.vector.tensor_copy(
                vt_v[:, m, :, DH:DH + 1],
                ones_t.rearrange("p (h o) -> p h o", o=1))
        wv_ts = [load_rounded([P, H], wv_d[k], "wvo", bufs=HK) for k in range(HK)]
        for m in range(SQ):
            ps = psAB.tile([P, H], F32, tag="big", bufs=2)
            for k in range(HK):
                nc.tensor.matmul(ps[:, 0:512], lhsT=xTr_sb[:, k, m * P:(m + 1) * P],
                                 rhs=wv_ts[k][:, 0:512],
                                 start=(k == 0), stop=(k == HK - 1))
            for k in range(HK):
                nc.tensor.matmul(ps[:, 512:H], lhsT=xTr_sb[:, k, m * P:(m + 1) * P],
                                 rhs=wv_ts[k][:, 512:H],
                                 start=(k == 0), stop=(k == HK - 1))
            dst = vt_sb.rearrange("p m (h c) -> p m h c", c=VW)[:, m, :, 0:DH]
            src = ps.rearrange("p (h d) -> p h d", d=DH)
            if use_bv:
                nc.vector.tensor_add(
                    src, src, bv_bc.rearrange("p (h d) -> p h d", d=DH))
            nc.vector.tensor_copy(dst, src)

        # ---- per-head attention ----
        for h in range(NH):
            mt, pb = h // 2, 64 * (h % 2)
            est = expp.tile([P, SQ, S], F32R, tag="est")
            for sk in range(SQ):
                ps_s = psAB.tile([P, S], F32, tag="s512", bufs=3)
                nc.tensor.matmul(
                    ps_s,
                    lhsT=kt_sb[pb:pb + DH, mt, sk * P:(sk + 1) * P],
                    rhs=qt_sb[pb:pb + DH, mt, :],
                    start=True, stop=True)
                if use_mask:
                    nc.scalar.activation(est[:, sk, :], ps_s, AF.Exp,
                                         bias=msk_sb[:, sk:sk + 1], scale=0.125)
                else:
                    nc.scalar.activation(est[:, sk, :], ps_s, AF.Exp, scale=0.125)
            ps_c = psAB.tile([P, S], F32, tag="ctx", bufs=1)
            for sk in range(SQ):
                nc.tensor.matmul(ps_c[0:VW, :],
                                 lhsT=vt_sb[:, sk, h * VW:(h + 1) * VW],
                                 rhs=est[:, sk, :],
                                 start=(sk == 0), stop=(sk == SQ - 1))
            nc.vector.tensor_copy(ctxt_sb[pb:pb + DH, mt, :], ps_c[0:DH, :])
            # softmax normalization of this head's ctxT rows: 1/sums
            # partition-broadcast on GpSimd (idle engine; exact on HW).
            rrow = smalls.tile([1, S], F32, tag="rrow")
            nc.vector.reciprocal(rrow, ps_c[DH:VW, :])
            rb = rbp.tile([P, S], F32, tag="rb")
            nc.gpsimd.partition_broadcast(rb, rrow)
            nc.vector.tensor_mul(ctxt_sb[pb:pb + DH, mt, :],
                                 ctxt_sb[pb:pb + DH, mt, :], rb[pb:pb + DH, :])

        # ---- Wo + residual + LN1 ; x1 transpose ----
        ph_att.close()
        with tc.tile_pool(name="psC", bufs=1, space="PSUM") as psC:
            wo_ts = [load_rounded([P, H], wo_d[k], "wvo", bufs=HK)
                     for k in range(HK)]
            for m in range(SQ):
                ps = psC.tile([P, H], F32, tag="cbig", bufs=2)
                for k in range(HK):
                    nc.tensor.matmul(ps[:, 0:512],
                                     lhsT=ctxt_sb[:, k, m * P:(m + 1) * P],
                                     rhs=wo_ts[k][:, 0:512],
                                     start=(k == 0), stop=(k == HK - 1))
                for k in range(HK):
                    nc.tensor.matmul(ps[:, 512:H],
                                     lhsT=ctxt_sb[:, k, m * P:(m + 1) * P],
                                     rhs=wo_ts[k][:, 512:H],
                                     start=(k == 0), stop=(k == HK - 1))
                a = a_pool.tile([P, H], F32, tag="a")
                nc.vector.tensor_add(a, ps, x_sb[:, m, :])
                if use_bo:
                    nc.vector.tensor_add(a, a, bo_bc)
                layernorm(a, x1_sb[:, m, :], ln1g_bc, ln1b_bc, use_ln1)
                for kb in range(HK):
                    ps_t = psC.tile([P, P], F32, tag="tr", bufs=3)
                    nc.tensor.transpose(
                        ps_t, x1_sb[:, m, kb * P:(kb + 1) * P], ident)
                    nc.vector.tensor_copy(
                        x1t_sb[:, kb, m * P:(m + 1) * P], ps_t)

    # ---- FFN ----
    with ExitStack() as phase_ffn:
        actF = phase_ffn.enter_context(tc.tile_pool(name="actF", bufs=1))
        hmidt_sb = actF.tile([P, FK, S], F32R)

        with tc.tile_pool(name="psD", bufs=4, space="PSUM") as psD:
            for m in range(FK):
                wi_t = load_rounded([P, HK, P], wi_d[m], "wsm")
                ps = psD.tile([P, S], F32, tag="f1")
                for k in range(HK):
                    nc.tensor.matmul(ps, lhsT=wi_t[:, k, :], rhs=x1t_sb[:, k, :],
                                     start=(k == 0), stop=(k == HK - 1))
                if use_bi:
                    nc.scalar.activation(hmidt_sb[:, m, :], ps, AF.Gelu,
                                         bias=bi_sb[:, m:m + 1])
                else:
                    nc.scalar.activation(hmidt_sb[:, m, :], ps, AF.Gelu)

        with tc.tile_pool(name="psE", bufs=1, space="PSUM") as psE:
            ps_m = [psE.tile([P, H], F32, tag=f"f2_{m}", bufs=1, name=f"psE{m}")
                    for m in range(SQ)]
            for k in range(FK):
                wo_t = load_rounded([P, H], wout_d[k], "wout", bufs=3)
                for m in range(SQ):
                    nc.tensor.matmul(ps_m[m][:, 0:512],
                                     lhsT=hmidt_sb[:, k, m * P:(m + 1) * P],
                                     rhs=wo_t[:, 0:512],
                                     start=(k == 0), stop=(k == FK - 1))
                    nc.tensor.matmul(ps_m[m][:, 512:H],
                                     lhsT=hmidt_sb[:, k, m * P:(m + 1) * P],
                                     rhs=wo_t[:, 512:H],
                                     start=(k == 0), stop=(k == FK - 1))
            for m in range(SQ):
                a = a_pool.tile([P, H], F32, tag="a")
                nc.vector.tensor_add(a, ps_m[m], x1_sb[:, m, :])
                if use_bout:
                    nc.vector.tensor_add(a, a, bout_bc)
                o_t = outp.tile([P, H], F32, tag="out")
                layernorm(a, o_t, ln2g_bc, ln2b_bc, use_ln2)
                nc.sync.dma_start(out=out_d[m * P:(m + 1) * P, :], in_=o_t)


_NC_CACHE = {}


def build_nc(flags):
    key = tuple(flags)
    if key not in _NC_CACHE:
        nc = bacc.Bacc("TRN2")
        with ExitStack() as ctx:
            tc = ctx.enter_context(tile.TileContext(nc))
            _emit(ctx, tc, flags)
        nc.compile()
        _NC_CACHE[key] = nc
    return _NC_CACHE[key]


def _pack_lhsT(A, mt):
    # A [in, mt*P] -> [mt, P, in//P, P] tiles: out[m, p, k, f] = A[P*k+p, P*m+f]
    kt = A.shape[0] // P
    return np.ascontiguousarray(
        A.reshape(kt, P, mt, P).transpose(2, 1, 0, 3))


def kernel(**inputs):
    hs = np.ascontiguousarray(np.asarray(inputs["hidden_states"], dtype=np.float32))
    eidx = np.asarray(inputs["expert_idx"]).astype(np.int64)
    mask = np.asarray(inputs["attention_mask"], dtype=np.float32)
    Wq = np.asarray(inputs["Wq"], dtype=np.float32)
    bq = np.asarray(inputs["bq"], dtype=np.float32)
    Wk = np.asarray(inputs["Wk"], dtype=np.float32)
    bk = np.asarray(inputs["bk"], dtype=np.float32)
    Wv = np.asarray(inputs["Wv"], dtype=np.float32)
    bv = np.asarray(inputs["bv"], dtype=np.float32)
    Wo = np.asarray(inputs["Wo"], dtype=np.float32)
    bo = np.asarray(inputs["bo"], dtype=np.float32)
    ln1_g = np.asarray(inputs["ln1_g"], dtype=np.float32)
    ln1_b = np.asarray(inputs["ln1_b"], dtype=np.float32)
    Wi = np.asarray(inputs["Wi"], dtype=np.float32)
    bi = np.asarray(inputs["bi"], dtype=np.float32)
    Wout = np.asarray(inputs["Wout"], dtype=np.float32)
    bout = np.asarray(inputs["bout"], dtype=np.float32)
    ln2_g = np.asarray(inputs["ln2_g"], dtype=np.float32)
    ln2_b = np.asarray(inputs["ln2_b"], dtype=np.float32)

    B = hs.shape[0]
    assert hs.shape == (B, S, H) and B == N_CORES

    use_bq = bool(np.any(bq))
    use_bk = bool(np.any(bk))
    use_bv = bool(np.any(bv))
    use_bo = bool(np.any(bo))
    use_bi = bool(np.any(bi))
    use_bout = bool(np.any(bout))
    use_mask = bool(np.any(mask))
    use_ln1 = bool(np.any(ln1_g != 1.0) or np.any(ln1_b))
    use_ln2 = bool(np.any(ln2_g != 1.0) or np.any(ln2_b))
    flags = (use_bq, use_bk, use_bv, use_bo, use_bi, use_bout,
             use_mask, use_ln1, use_ln2)

    nc = build_nc(flags)

    in_maps = []
    for b in range(B):
        e = int(eidx[b])
        xb = hs[b]
        im = {
            "x": xb,
            "xT": np.ascontiguousarray(xb.T),
            "wq": _pack_lhsT(Wq[e], HK),
            "wk": _pack_lhsT(Wk[e], HK),
            "wv": np.ascontiguousarray(Wv[e].reshape(HK, P, H)),
            "wo": np.ascontiguousarray(Wo[e].reshape(HK, P, H)),
            "wi": _pack_lhsT(Wi[e], FK),
            "wout": np.ascontiguousarray(Wout[e].reshape(FK, P, H)),
        }
        if use_bq:
            im["bq"] = np.ascontiguousarray(bq[e].reshape(HK, P).T)
        if use_bk:
            im["bk"] = np.ascontiguousarray(bk[e].reshape(HK, P).T)
        if use_bv:
            im["bv"] = bv[e]
        if use_bo:
            im["bo"] = bo[e]
        if use_bi:
            im["bi"] = np.ascontiguousarray(bi[e].reshape(FK, P).T)
        if use_bout:
            im["bout"] = bout[e]
        if use_mask:
            im["msk"] = np.ascontiguousarray(mask[b, 0, 0, :].reshape(SQ, P).T)
        if use_ln1:
            im["ln1g"] = ln1_g
            im["ln1b"] = ln1_b
        if use_ln2:
            im["ln2g"] = ln2_g
            im["ln2b"] = ln2_b
        in_maps.append(im)

    from concourse.bass_utils import run_bass_kernel_spmd
    res = run_bass_kernel_spmd(nc, in_maps, core_ids=list(range(N_CORES)),
                               **RUN_KWARGS)
    global LAST_RESULTS
    LAST_RESULTS = res
    out = np.stack([res.results[b]["out"] for b in range(B)], axis=0)
    return out.astype(np.float32)


RUN_KWARGS = {}
LAST_RESULTS = None


if __name__ == "__main__":
    rng = np.random.default_rng(0)
    demo = {
        "hidden_states": rng.standard_normal((8, S, H), dtype=np.float32),
        "expert_idx": rng.integers(0, 4, size=8).astype(np.int32),
        "attention_mask": np.zeros((8, 1, 1, S), np.float32),
        "Wq": 0.02 * rng.standard_normal((4, H, H), dtype=np.float32),
        "bq": np.zeros((4, H), np.float32),
        "Wk": 0.02 * rng.standard_normal((4, H, H), dtype=np.float32),
        "bk": np.zeros((4, H), np.float32),
        "Wv": 0.02 * rng.standard_normal((4, H, H), dtype=np.float32),
        "bv": np.zeros((4, H), np.float32),
        "Wo": 0.02 * rng.standard_normal((4, H, H), dtype=np.float32),
        "bo": np.zeros((4, H), np.float32),
        "ln1_g": np.ones((H,), np.float32),
        "ln1_b": np.zeros((H,), np.float32),
        "Wi": 0.02 * rng.standard_normal((4, H, FF), dtype=np.float32),
        "bi": np.zeros((4, FF), np.float32),
        "Wout": 0.02 * rng.standard_normal((4, FF, H), dtype=np.float32),
        "bout": np.zeros((4, H), np.float32),
        "ln2_g": np.ones((H,), np.float32),
        "ln2_b": np.zeros((H,), np.float32),
    }
    out = kernel(**demo)
    print("out", out.shape, out.dtype, float(np.abs(out).mean()))


# revision 39
# speedup vs baseline: 2.1511x; 1.0203x over previous
"""MoE BERT layer (nn_MoEBertLayer) on 8 Trainium2 NeuronCores.

Sharding: pure data parallel. B=8 samples -> 1 sample per core. The MoE
routing (per-sample expert selection) is done on the host: each core's
input map carries the weights of the expert its sample routed to, packed
into matmul-friendly tile layouts. The device kernel is a dense BERT
layer for a single [512, 768] sample. No collectives.

Kernel layout strategy (per core, S=512, H=768, FF=3072, NH=12, DH=64):
  - hidden_states shipped in both [S,H] (residual/LN side) and [H,S]
    (matmul lhsT side) layouts.
  - QT/KT computed directly in [H,S] layout (out = Wq[:,m]^T @ xT).
  - V computed in [S,H] layout, with a constant ones column appended per
    head (width 65): the attention-context matmul
    ctxU_h^T = [V_h | 1]^T @ exp(scores_h^T) then yields the softmax
    denominator as its last row for free.
  - scores_h^T = K_h Q_h^T computed per head ([Sk,Sq] layout), exp via
    ScalarE with the 1/sqrt(DH) scale fused; softmax normalization is
    applied to ctxU^T (768x512 elements instead of 12x512x512).
  - Wo/FFN2 outputs come out in [S,H] layout where LayerNorm reductions
    are free-dim reductions (bn_stats/bn_aggr).
  - One on-chip transpose x1 -> x1T (24 PE transposes) feeds the FFN.
"""

import os
import sys
import numpy as np
from contextlib import ExitStack

for _p in ("/opt/trn_rl_repo", os.path.expanduser("~/.axon_site/_ro/trn_rl_repo")):
    if os.path.isdir(_p) and _p not in sys.path:
        sys.path.append(_p)

import concourse.bass as bass
import concourse.bacc as bacc
import concourse.tile as tile
from concourse import mybir
from concourse.masks import make_identity

F32 = mybir.dt.float32
F32R = mybir.dt.float32r
AF = mybir.ActivationFunctionType

P = 128
S = 512           # sequence length (per sample)
H = 768           # hidden size
FF = 3072         # FFN intermediate
NH = 12           # attention heads
DH = 64           # head dim
HK = H // P       # 6
SQ = S // P       # 4
FK = FF // P      # 24
VW = DH + 1       # 65: V head block + ones column
N_CORES = 8
EPS = 1e-12


def _emit(ctx, tc, flags):
    nc = tc.nc
    (use_bq, use_bk, use_bv, use_bo, use_bi, use_bout,
     use_mask, use_ln1, use_ln2) = flags

    xT_d = nc.dram_tensor("xT", [H, S], F32, kind="ExternalInput")
    x_d = nc.dram_tensor("x", [S, H], F32, kind="ExternalInput")
    wq_d = nc.dram_tensor("wq", [HK, P, HK, P], F32, kind="ExternalInput")
    wk_d = nc.dram_tensor("wk", [HK, P, HK, P], F32, kind="ExternalInput")
    wv_d = nc.dram_tensor("wv", [HK, P, H], F32, kind="ExternalInput")
    wo_d = nc.dram_tensor("wo", [HK, P, H], F32, kind="ExternalInput")
    wi_d = nc.dram_tensor("wi", [FK, P, HK, P], F32, kind="ExternalInput")
    wout_d = nc.dram_tensor("wout", [FK, P, H], F32, kind="ExternalInput")
    out_d = nc.dram_tensor("out", [S, H], F32, kind="ExternalOutput")

    # optional inputs (general path; absent in the fast path)
    bq_d = nc.dram_tensor("bq", [P, HK], F32, kind="ExternalInput") if use_bq else None
    bk_d = nc.dram_tensor("bk", [P, HK], F32, kind="ExternalInput") if use_bk else None
    bv_d = nc.dram_tensor("bv", [H], F32, kind="ExternalInput") if use_bv else None
    bo_d = nc.dram_tensor("bo", [H], F32, kind="ExternalInput") if use_bo else None
    bi_d = nc.dram_tensor("bi", [P, FK], F32, kind="ExternalInput") if use_bi else None
    bout_d = nc.dram_tensor("bout", [H], F32, kind="ExternalInput") if use_bout else None
    msk_d = nc.dram_tensor("msk", [P, SQ], F32, kind="ExternalInput") if use_mask else None
    ln1g_d = nc.dram_tensor("ln1g", [H], F32, kind="ExternalInput") if use_ln1 else None
    ln1b_d = nc.dram_tensor("ln1b", [H], F32, kind="ExternalInput") if use_ln1 else None
    ln2g_d = nc.dram_tensor("ln2g", [H], F32, kind="ExternalInput") if use_ln2 else None
    ln2b_d = nc.dram_tensor("ln2b", [H], F32, kind="ExternalInput") if use_ln2 else None

    def bcast_dram_row(dram_ap, parts=P):
        # DRAM [N] -> partition-broadcast [parts, N] AP for DMA
        return bass.AP(tensor=dram_ap.tensor, offset=dram_ap.offset,
                       ap=[[0, parts]] + list(dram_ap.ap))

    # ---------------- pools: whole-kernel lifetime ----------------
    const = ctx.enter_context(tc.tile_pool(name="const", bufs=1))
    outp = ctx.enter_context(tc.tile_pool(name="outp", bufs=2))
    wsmall = ctx.enter_context(tc.tile_pool(name="wsmall", bufs=4))
    wbig = ctx.enter_context(tc.tile_pool(name="wbig", bufs=6))
    wraw = ctx.enter_context(tc.tile_pool(name="wraw", bufs=3))
    smalls = ctx.enter_context(tc.tile_pool(name="smalls", bufs=4))

    # All matmul operands are float32r (single-pass PE mode, ~2^-12
    # rounding, 4x faster than true fp32). The BIR verifier requires a
    # rounding-capable producer, so DMA'd tensors go through a GpSimd
    # (otherwise idle) round-copy; on-chip operands are written as f32r
    # directly by their eviction op.
    def load_rounded(shape, dram_ap, tag, bufs=None, eng=None):
        raw = wraw.tile(shape, F32, tag="wraw", name="wraw")
        nc.sync.dma_start(out=raw, in_=dram_ap)
        pool = wsmall if shape[-1] == P else wbig
        t = pool.tile(shape, F32R, tag=tag, name=tag, bufs=bufs)
        (eng or nc.vector).tensor_copy(t, raw)
        return t

    ident = const.tile([P, P], F32)
    make_identity(nc, ident)
    eps_t = const.tile([P, 1], F32)
    nc.vector.memset(eps_t, EPS)

    bq_sb = bk_sb = bi_sb = None
    bv_bc = bo_bc = bout_bc = msk_sb = None
    ln1g_bc = ln1b_bc = ln2g_bc = ln2b_bc = None
    if use_bq:
        bq_sb = const.tile([P, HK], F32)
        nc.sync.dma_start(out=bq_sb, in_=bq_d[:])
    if use_bk:
        bk_sb = const.tile([P, HK], F32)
        nc.sync.dma_start(out=bk_sb, in_=bk_d[:])
    if use_bi:
        bi_sb = const.tile([P, FK], F32)
        nc.sync.dma_start(out=bi_sb, in_=bi_d[:])
    if use_bv:
        bv_bc = const.tile([P, H], F32)
        nc.sync.dma_start(out=bv_bc, in_=bcast_dram_row(bv_d[:]))
    if use_bo:
        bo_bc = const.tile([P, H], F32)
        nc.sync.dma_start(out=bo_bc, in_=bcast_dram_row(bo_d[:]))
    if use_bout:
        bout_bc = const.tile([P, H], F32)
        nc.sync.dma_start(out=bout_bc, in_=bcast_dram_row(bout_d[:]))
    if use_mask:
        msk_sb = const.tile([P, SQ], F32)
        nc.sync.dma_start(out=msk_sb, in_=msk_d[:])
    if use_ln1:
        ln1g_bc = const.tile([P, H], F32)
        nc.sync.dma_start(out=ln1g_bc, in_=bcast_dram_row(ln1g_d[:]))
        ln1b_bc = const.tile([P, H], F32)
        nc.sync.dma_start(out=ln1b_bc, in_=bcast_dram_row(ln1b_d[:]))
    if use_ln2:
        ln2g_bc = const.tile([P, H], F32)
        nc.sync.dma_start(out=ln2g_bc, in_=bcast_dram_row(ln2g_d[:]))
        ln2b_bc = const.tile([P, H], F32)
        nc.sync.dma_start(out=ln2b_bc, in_=bcast_dram_row(ln2b_d[:]))

    # layer-norm core: reads `a` [P,H] (SBUF), writes `dst` [P,H]
    def layernorm(a, dst, g_bc, b_bc, use_gb):
        st = smalls.tile([P, 3, 6], F32, tag="lnst")
        a3 = a.rearrange("p (n f) -> p n f", f=256)
        for sg in range(3):
            nc.vector.bn_stats(out=st[:, sg, :], in_=a3[:, sg, :])
        mv = smalls.tile([P, 2], F32, tag="lnmv")
        nc.vector.bn_aggr(out=mv, in_=st)
        sd = smalls.tile([P, 1], F32, tag="lnsd")
        nc.scalar.activation(sd, mv[:, 1:2], AF.Sqrt, bias=eps_t)
        rsig = smalls.tile([P, 1], F32, tag="lnrs")
        nc.vector.reciprocal(rsig, sd)
        nm = smalls.tile([P, 1], F32, tag="lnnm")
        nc.vector.tensor_mul(nm, mv[:, 0:1], rsig)
        nc.vector.tensor_scalar_mul(nm, nm, -1.0)
        nc.scalar.activation(dst, a, AF.Identity, bias=nm, scale=rsig)
        if use_gb:
            nc.vector.tensor_mul(dst, dst, g_bc)
            nc.vector.tensor_add(dst, dst, b_bc)

    # ---------------- mid-lifetime activations ----------------
    act1 = ctx.enter_context(tc.tile_pool(name="act1", bufs=1))
    x1_sb = act1.tile([P, SQ, H], F32)      # LN1 output [S,H]
    x1t_sb = act1.tile([P, HK, S], F32R)    # x1 transposed [H,S]

    a_pool = ctx.enter_context(tc.tile_pool(name="a_pool", bufs=2))

    with ExitStack() as phase_ab:
        actA = phase_ab.enter_context(tc.tile_pool(name="actA", bufs=1))
        x_sb = actA.tile([P, SQ, H], F32)
        xTr_sb = actA.tile([P, HK, S], F32R)
        qt_sb = actA.tile([P, HK, S], F32R)
        kt_sb = actA.tile([P, HK, S], F32R)
        vt_sb = actA.tile([P, SQ, NH * VW], F32R)
        ctxt_sb = actA.tile([P, HK, S], F32R)

        for m in range(HK):
            raw = wraw.tile([P, S], F32, tag="wraw", name="wraw")
            nc.sync.dma_start(out=raw, in_=xT_d[m * P:(m + 1) * P, :])
            nc.vector.tensor_copy(xTr_sb[:, m, :], raw)
        ph_att = phase_ab.enter_context(ExitStack())
        psAB = ph_att.enter_context(tc.tile_pool(name="psAB", bufs=1, space="PSUM"))
        expp = ph_att.enter_context(tc.tile_pool(name="expp", bufs=2))
        rbp = ph_att.enter_context(tc.tile_pool(name="rbp", bufs=2))

        # ---- QT / KT:  out[m] = W[:, m-block]^T @ xT  ([H,S] layout) ----
        for (w_d, dst, b_sb, useb) in ((wq_d, qt_sb, bq_sb, use_bq),
                                       (wk_d, kt_sb, bk_sb, use_bk)):
            for m in range(HK):
                w_t = load_rounded([P, HK, P], w_d[m], "wsm")
                ps = psAB.tile([P, S], F32, tag="ctx", bufs=2, name="psqk")
                for k in range(HK):
                    nc.tensor.matmul(ps, lhsT=w_t[:, k, :], rhs=xTr_sb[:, k, :],
                                     start=(k == 0), stop=(k == HK - 1))
                if useb:
                    nc.scalar.activation(dst[:, m, :], ps, AF.Identity,
                                         bias=b_sb[:, m:m + 1])
                else:
                    nc.vector.tensor_copy(dst[:, m, :], ps)

        for m in range(SQ):
            nc.sync.dma_start(out=x_sb[:, m, :], in_=x_d[m * P:(m + 1) * P, :])

        # ---- V in [S,H] layout with ones column per head -> vt_sb ----
        ones_t = const.tile([P, NH], F32)
        nc.vector.memset(ones_t, 1.0)
        vt_v = vt_sb.rearrange("p m (h c) -> p m h c", c=VW)
        for m in range(SQ):
            nc.vector.tensor_copy(
                vt_v[:, m, :, DH:DH + 1],
                ones_t.rearrange("p (h o) -> p h o", o=1))
        wv_ts = [load_rounded([P, H], wv_d[k], "wvo", bufs=HK) for k in range(HK)]
        for m in range(SQ):
            ps = psAB.tile([P, H], F32, tag="big", bufs=1)
            for k in range(HK):
                nc.tensor.matmul(ps[:, 0:512], lhsT=xTr_sb[:, k, m * P:(m + 1) * P],
                                 rhs=wv_ts[k][:, 0:512],
                                 start=(k == 0), stop=(k == HK - 1))
            for k in range(HK):
                nc.tensor.matmul(ps[:, 512:H], lhsT=xTr_sb[:, k, m * P:(m + 1) * P],
                                 rhs=wv_ts[k][:, 512:H],
                                 start=(k == 0), stop=(k == HK - 1))
            dst = vt_sb.rearrange("p m (h c) -> p m h c", c=VW)[:, m, :, 0:DH]
            src = ps.rearrange("p (h d) -> p h d", d=DH)
            if use_bv:
                nc.vector.tensor_add(
                    src, src, bv_bc.rearrange("p (h d) -> p h d", d=DH))
            nc.vector.tensor_copy(dst, src)

        # ---- attention, head pairs: the two heads of a pair live at
        # partition bases 0/64 of the same kt/qt tile, so their score
        # matmuls (K=64) row-pack onto disjoint PE row groups and run
        # concurrently; both score outputs share one 2-bank psum tile so
        # exp processes 1024 columns per ACT op. ----
        for hp in range(NH // 2):
            mt = hp
            est = expp.tile([P, SQ, 2 * S], F32R, tag="est")
            for sk in range(SQ):
                ps_s = psAB.tile([P, 2 * S], F32, tag="spair", bufs=2)
                for half in range(2):
                    pb = 64 * half
                    nc.tensor.matmul(
                        ps_s[:, half * S:(half + 1) * S],
                        lhsT=kt_sb[pb:pb + DH, mt, sk * P:(sk + 1) * P],
                        rhs=qt_sb[pb:pb + DH, mt, :],
                        start=True, stop=True)
                if use_mask:
                    nc.scalar.activation(est[:, sk, :], ps_s, AF.Exp,
                                         bias=msk_sb[:, sk:sk + 1], scale=0.125)
                else:
                    nc.scalar.activation(est[:, sk, :], ps_s, AF.Exp, scale=0.125)
            for half in range(2):
                h = 2 * hp + half
                pb = 64 * half
                ps_c = psAB.tile([P, S], F32, tag="ctx", bufs=2, name="psc")
                for sk in range(SQ):
                    nc.tensor.matmul(ps_c[0:VW, :],
                                     lhsT=vt_sb[:, sk, h * VW:(h + 1) * VW],
                                     rhs=est[:, sk, half * S:(half + 1) * S],
                                     start=(sk == 0), stop=(sk == SQ - 1))
                nc.vector.tensor_copy(ctxt_sb[pb:pb + DH, mt, :], ps_c[0:DH, :])
                # softmax normalization of this head's ctxT rows: 1/sums
                # partition-broadcast on GpSimd (idle engine; exact on HW).
                rrow = smalls.tile([1, S], F32, tag="rrow")
                nc.vector.reciprocal(rrow, ps_c[DH:VW, :])
                rb = rbp.tile([P, S], F32, tag="rb")
                nc.gpsimd.partition_broadcast(rb, rrow)
                nc.vector.tensor_mul(ctxt_sb[pb:pb + DH, mt, :],
                                     ctxt_sb[pb:pb + DH, mt, :], rb[pb:pb + DH, :])

        # ---- Wo + residual + LN1 ; x1 transpose ----
        ph_att.close()
        with tc.tile_pool(name="psC", bufs=1, space="PSUM") as psC:
            wo_ts = [load_rounded([P, H], wo_d[k], "wvo", bufs=HK)
                     for k in range(HK)]
            for m in range(SQ):
                ps = psC.tile([P, H], F32, tag="cbig", bufs=2)
                for k in range(HK):
                    nc.tensor.matmul(ps[:, 0:512],
                                     lhsT=ctxt_sb[:, k, m * P:(m + 1) * P],
                                     rhs=wo_ts[k][:, 0:512],
                                     start=(k == 0), stop=(k == HK - 1))
                for k in range(HK):
                    nc.tensor.matmul(ps[:, 512:H],
                                     lhsT=ctxt_sb[:, k, m * P:(m + 1) * P],
                                     rhs=wo_ts[k][:, 512:H],
                                     start=(k == 0), stop=(k == HK - 1))
                a = a_pool.tile([P, H], F32, tag="a")
                nc.vector.tensor_add(a, ps, x_sb[:, m, :])
                if use_bo:
                    nc.vector.tensor_add(a, a, bo_bc)
                layernorm(a, x1_sb[:, m, :], ln1g_bc, ln1b_bc, use_ln1)
            for m in range(SQ):
                for kb in range(HK):
                    ps_t = psC.tile([P, P], F32, tag="tr", bufs=3)
                    nc.tensor.transpose(
                        ps_t, x1_sb[:, m, kb * P:(kb + 1) * P], ident)
                    nc.vector.tensor_copy(
                        x1t_sb[:, kb, m * P:(m + 1) * P], ps_t)

    # ---- FFN ----
    with ExitStack() as phase_ffn:
        actF = phase_ffn.enter_context(tc.tile_pool(name="actF", bufs=1))
        hmidt_sb = actF.tile([P, FK, S], F32R)

        with tc.tile_pool(name="psD", bufs=4, space="PSUM") as psD:
            for m in range(FK):
                wi_t = load_rounded([P, HK, P], wi_d[m], "wsm")
                ps = psD.tile([P, S], F32, tag="f1")
                for k in range(HK):
                    nc.tensor.matmul(ps, lhsT=wi_t[:, k, :], rhs=x1t_sb[:, k, :],
                                     start=(k == 0), stop=(k == HK - 1))
                if use_bi:
                    nc.scalar.activation(hmidt_sb[:, m, :], ps, AF.Gelu,
                                         bias=bi_sb[:, m:m + 1])
                else:
                    nc.scalar.activation(hmidt_sb[:, m, :], ps, AF.Gelu)

        with tc.tile_pool(name="psE", bufs=1, space="PSUM") as psE:
            ps_m = [psE.tile([P, H], F32, tag=f"f2_{m}", bufs=1, name=f"psE{m}")
                    for m in range(SQ)]
            for k in range(FK):
                wo_t = load_rounded([P, H], wout_d[k], "wout", bufs=3)
                for m in range(SQ):
                    nc.tensor.matmul(ps_m[m][:, 0:512],
                                     lhsT=hmidt_sb[:, k, m * P:(m + 1) * P],
                                     rhs=wo_t[:, 0:512],
                                     start=(k == 0), stop=(k == FK - 1))
                    nc.tensor.matmul(ps_m[m][:, 512:H],
                                     lhsT=hmidt_sb[:, k, m * P:(m + 1) * P],
                                     rhs=wo_t[:, 512:H],
                                     start=(k == 0), stop=(k == FK - 1))
            for m in range(SQ):
                a = a_pool.tile([P, H], F32, tag="a")
                nc.vector.tensor_add(a, ps_m[m], x1_sb[:, m, :])
                if use_bout:
                    nc.vector.tensor_add(a, a, bout_bc)
                o_t = outp.tile([P, H], F32, tag="out")
                layernorm(a, o_t, ln2g_bc, ln2b_bc, use_ln2)
                nc.sync.dma_start(out=out_d[m * P:(m + 1) * P, :], in_=o_t)


_NC_CACHE = {}


def build_nc(flags):
    key = tuple(flags)
    if key not in _NC_CACHE:
        nc = bacc.Bacc("TRN2")
        with ExitStack() as ctx:
            tc = ctx.enter_context(tile.TileContext(nc))
            _emit(ctx, tc, flags)
        nc.compile()
        _NC_CACHE[key] = nc
    return _NC_CACHE[key]


def _pack_lhsT(A, mt):
    # A [in, mt*P] -> [mt, P, in//P, P] tiles: out[m, p, k, f] = A[P*k+p, P*m+f]
    kt = A.shape[0] // P
    return np.ascontiguousarray(
        A.reshape(kt, P, mt, P).transpose(2, 1, 0, 3))


def kernel(**inputs):
    hs = np.ascontiguousarray(np.asarray(inputs["hidden_states"], dtype=np.float32))
    eidx = np.asarray(inputs["expert_idx"]).astype(np.int64)
    mask = np.asarray(inputs["attention_mask"], dtype=np.float32)
    Wq = np.asarray(inputs["Wq"], dtype=np.float32)
    bq = np.asarray(inputs["bq"], dtype=np.float32)
    Wk = np.asarray(inputs["Wk"], dtype=np.float32)
    bk = np.asarray(inputs["bk"], dtype=np.float32)
    Wv = np.asarray(inputs["Wv"], dtype=np.float32)
    bv = np.asarray(inputs["bv"], dtype=np.float32)
    Wo = np.asarray(inputs["Wo"], dtype=np.float32)
    bo = np.asarray(inputs["bo"], dtype=np.float32)
    ln1_g = np.asarray(inputs["ln1_g"], dtype=np.float32)
    ln1_b = np.asarray(inputs["ln1_b"], dtype=np.float32)
    Wi = np.asarray(inputs["Wi"], dtype=np.float32)
    bi = np.asarray(inputs["bi"], dtype=np.float32)
    Wout = np.asarray(inputs["Wout"], dtype=np.float32)
    bout = np.asarray(inputs["bout"], dtype=np.float32)
    ln2_g = np.asarray(inputs["ln2_g"], dtype=np.float32)
    ln2_b = np.asarray(inputs["ln2_b"], dtype=np.float32)

    B = hs.shape[0]
    assert hs.shape == (B, S, H) and B == N_CORES

    use_bq = bool(np.any(bq))
    use_bk = bool(np.any(bk))
    use_bv = bool(np.any(bv))
    use_bo = bool(np.any(bo))
    use_bi = bool(np.any(bi))
    use_bout = bool(np.any(bout))
    use_mask = bool(np.any(mask))
    use_ln1 = bool(np.any(ln1_g != 1.0) or np.any(ln1_b))
    use_ln2 = bool(np.any(ln2_g != 1.0) or np.any(ln2_b))
    flags = (use_bq, use_bk, use_bv, use_bo, use_bi, use_bout,
             use_mask, use_ln1, use_ln2)

    nc = build_nc(flags)

    in_maps = []
    for b in range(B):
        e = int(eidx[b])
        xb = hs[b]
        im = {
            "x": xb,
            "xT": np.ascontiguousarray(xb.T),
            "wq": _pack_lhsT(Wq[e], HK),
            "wk": _pack_lhsT(Wk[e], HK),
            "wv": np.ascontiguousarray(Wv[e].reshape(HK, P, H)),
            "wo": np.ascontiguousarray(Wo[e].reshape(HK, P, H)),
            "wi": _pack_lhsT(Wi[e], FK),
            "wout": np.ascontiguousarray(Wout[e].reshape(FK, P, H)),
        }
        if use_bq:
            im["bq"] = np.ascontiguousarray(bq[e].reshape(HK, P).T)
        if use_bk:
            im["bk"] = np.ascontiguousarray(bk[e].reshape(HK, P).T)
        if use_bv:
            im["bv"] = bv[e]
        if use_bo:
            im["bo"] = bo[e]
        if use_bi:
            im["bi"] = np.ascontiguousarray(bi[e].reshape(FK, P).T)
        if use_bout:
            im["bout"] = bout[e]
        if use_mask:
            im["msk"] = np.ascontiguousarray(mask[b, 0, 0, :].reshape(SQ, P).T)
        if use_ln1:
            im["ln1g"] = ln1_g
            im["ln1b"] = ln1_b
        if use_ln2:
            im["ln2g"] = ln2_g
            im["ln2b"] = ln2_b
        in_maps.append(im)

    from concourse.bass_utils import run_bass_kernel_spmd
    res = run_bass_kernel_spmd(nc, in_maps, core_ids=list(range(N_CORES)),
                               **RUN_KWARGS)
    global LAST_RESULTS
    LAST_RESULTS = res
    out = np.stack([res.results[b]["out"] for b in range(B)], axis=0)
    return out.astype(np.float32)


RUN_KWARGS = {}
LAST_RESULTS = None


if __name__ == "__main__":
    rng = np.random.default_rng(0)
    demo = {
        "hidden_states": rng.standard_normal((8, S, H), dtype=np.float32),
        "expert_idx": rng.integers(0, 4, size=8).astype(np.int32),
        "attention_mask": np.zeros((8, 1, 1, S), np.float32),
        "Wq": 0.02 * rng.standard_normal((4, H, H), dtype=np.float32),
        "bq": np.zeros((4, H), np.float32),
        "Wk": 0.02 * rng.standard_normal((4, H, H), dtype=np.float32),
        "bk": np.zeros((4, H), np.float32),
        "Wv": 0.02 * rng.standard_normal((4, H, H), dtype=np.float32),
        "bv": np.zeros((4, H), np.float32),
        "Wo": 0.02 * rng.standard_normal((4, H, H), dtype=np.float32),
        "bo": np.zeros((4, H), np.float32),
        "ln1_g": np.ones((H,), np.float32),
        "ln1_b": np.zeros((H,), np.float32),
        "Wi": 0.02 * rng.standard_normal((4, H, FF), dtype=np.float32),
        "bi": np.zeros((4, FF), np.float32),
        "Wout": 0.02 * rng.standard_normal((4, FF, H), dtype=np.float32),
        "bout": np.zeros((4, H), np.float32),
        "ln2_g": np.ones((H,), np.float32),
        "ln2_b": np.zeros((H,), np.float32),
    }
    out = kernel(**demo)
    print("out", out.shape, out.dtype, float(np.abs(out).mean()))
